# revision 20
# baseline (speedup 1.0000x reference)
"""Trainium2 Bass kernel for nn_HardcodedKVMemoryBlock (8 NeuronCores).

Sharding: core i handles batch b=i//4, sequence chunk c=i%4 (512 tokens).
The (B,L,P,D) cumsum is restructured as causal linear attention:
    retrieved = tril(Q K^T) @ V + Q @ S_prefix
with Q=[cos,sin] phasors (L x 64), V = values at odd positions, and the
cross-chunk carry S_prefix = (K_even^T @ x_odd) @ val_W computed
redundantly per core from a zero-padded prefix (no collectives).
The 1/sqrt(valid*P) normalization cancels inside LayerNorm1 (scale
invariance); ln gains are folded into W1/Wo and means are applied as
rank-1 PE updates, so LN costs no extra full-width element-wise passes
beyond one multiply by the broadcast rstd.
"""

import math
import numpy as np
import ml_dtypes

import concourse.bass as bass
import concourse.tile as tile
from concourse import bacc, mybir
from concourse.bass_utils import run_bass_kernel_spmd

PI = math.pi
B, L, D, P = 2, 2048, 256, 32
T = 512          # own tokens per core
H = 512          # MLP hidden
PRE = 768        # padded prefix pair count (max prefix 1536 tokens / 2)
N_CORES = 8

f32 = mybir.dt.float32
f32r = mybir.dt.float32r
bf16 = mybir.dt.bfloat16
AF = mybir.ActivationFunctionType
OP = mybir.AluOpType


def _r(ap):
    return ap.bitcast(f32r)


def _build():
    nc = bacc.Bacc("TRN2", target_bir_lowering=False, debug=False,
                   num_devices=N_CORES)

    def din(name, shape, dt):
        return nc.dram_tensor(name, shape, dt, kind="ExternalInput").ap()

    xc = din("xc", [T, D], f32)
    xpe = din("xpe", [PRE, D], bf16)
    xpo = din("xpo", [PRE, D], bf16)
    kw = din("kw", [D, P], f32r)
    kwb = din("kwb", [D, P], bf16)
    kbc = din("kbc", [P, 1], f32)
    vw = din("vw", [D, D], f32r)
    w1g = din("w1g", [D, H], f32r)
    c1n = din("c1n", [1, H], f32r)
    cb1 = din("cb1", [H, 1], f32)
    w2 = din("w2", [H, D], f32r)
    b2c = din("b2c", [D, 1], f32)
    wog = din("wog", [D, D], f32r)
    c1on = din("c1on", [1, D], f32r)
    m0 = din("m0", [128, 256], f32)
    onesr = din("onesr", [1, 128], f32r)
    invd = din("invd", [128, 1], f32r)
    eyef = din("eyef", [128, 128], f32)
    eyeb = din("eyeb", [128, 128], bf16)
    outc = nc.dram_tensor("outc", [T, D], f32, kind="ExternalOutput").ap()

    with tile.TileContext(nc) as tc:
        _emit(tc, locals())
    nc.compile()
    return nc


def _emit(tc, io):
    nc = tc.nc
    xc, xpe, xpo = io["xc"], io["xpe"], io["xpo"]
    outc = io["outc"]

    sb = tc.alloc_tile_pool(name="sb", bufs=1)
    pt = tc.alloc_tile_pool(name="pt", bufs=2, space="PSUM")    # transposes
    pa = tc.alloc_tile_pool(name="pa", bufs=2, space="PSUM")    # narrow tiles
    pb = tc.alloc_tile_pool(name="pb", bufs=3, space="PSUM")    # [128,512]

    # ---------------- constant / weight tiles ----------------
    kw_sb = sb.tile([128, 2 * P], f32r)       # ktile kt at cols [P*kt]
    kwb_sb = sb.tile([128, 2 * P], bf16)
    kbc_sb = sb.tile([P, 1], f32)
    vw_sb = sb.tile([128, 512], f32r)         # ktile kt at cols [256*kt]
    w1g_sb = sb.tile([128, 1024], f32r)      # ktile kt at cols [512*kt]
    c1n_sb = sb.tile([1, H], f32r)
    cb1_sb = sb.tile([128, 4], f32)          # mtile m at col m
    w2_sb = sb.tile([128, 1024], f32r)       # ktile kt at cols [256*kt]
    b2c_sb = sb.tile([128, 2], f32)
    wog_sb = sb.tile([128, 512], f32r)       # ktile dh at cols [256*dh]
    c1on_sb = sb.tile([1, D], f32r)
    m0_sb = sb.tile([128, 256], f32)
    eyef_sb = sb.tile([128, 128], f32)
    eyeb_sb = sb.tile([128, 128], bf16)
    ones_sb = sb.tile([1, 128], f32r)
    invd_sb = sb.tile([128, 1], f32r)
    epsb_sb = sb.tile([128, 1], f32)
    halfpi_sb = sb.tile([P, 1], f32)
    zerop_sb = sb.tile([P, 1], f32)
    zero128_sb = sb.tile([128, 1], f32)
    zero1_sb = sb.tile([1, 1], f32)

    dma = nc.sync.dma_start
    dmaw = nc.gpsimd.dma_start
    dma(eyef_sb[:], io["eyef"])
    dma(eyeb_sb[:], io["eyeb"])
    dma(kw_sb[:], io["kw"].rearrange("(k p) q -> p k q", p=128))
    dma(kwb_sb[:], io["kwb"].rearrange("(k p) q -> p k q", p=128))
    dma(kbc_sb[:], io["kbc"])
    dmaw(vw_sb[:], io["vw"].rearrange("(k p) q -> p k q", p=128))
    dmaw(w1g_sb[:], io["w1g"].rearrange("(k p) q -> p k q", p=128))
    dmaw(c1n_sb[:], io["c1n"])
    dmaw(cb1_sb[:], io["cb1"].rearrange("(m p) o -> p m o", p=128))
    dmaw(w2_sb[:], io["w2"].rearrange("(k p) q -> p k q", p=128))
    dmaw(b2c_sb[:], io["b2c"].rearrange("(m p) o -> p m o", p=128))
    dmaw(wog_sb[:], io["wog"].rearrange("(k p) q -> p k q", p=128))
    dmaw(c1on_sb[:], io["c1on"])
    dmaw(m0_sb[:], io["m0"])
    dma(ones_sb[:], io["onesr"])
    dma(invd_sb[:], io["invd"])
    nc.vector.memset(epsb_sb[:], 1e-5)
    nc.vector.memset(halfpi_sb[:], PI / 2)
    nc.vector.memset(zerop_sb[:], 0.0)
    nc.vector.memset(zero128_sb[:], 0.0)
    nc.vector.memset(zero1_sb[:], 0.0)

    # ---------------- data tiles ----------------
    xc_sb = sb.tile([128, 1024], f32)        # token tile tt at cols [256*tt]
    xT_sb = sb.tile([128, 1024], f32r)        # d-half dh at cols [512*dh]
    xpo_sb = sb.tile([128, 1536], bf16)
    xpeT_sb = sb.tile([128, 1536], bf16)     # ktile kt at cols [768*kt]
    t_sb = sb.tile([P, T], f32)
    ta_sb = sb.tile([P, T], f32)
    qb_sb = sb.tile([2 * P, T], f32r)         # rows 0:32 cos, 32:64 sin
    tpre_sb = sb.tile([P, PRE], f32)
    tpa_sb = sb.tile([P, PRE], f32)
    kpre_sb = sb.tile([2 * P, PRE], bf16)
    kpreT_sb = sb.tile([128, 6 * 64], bf16)  # block j at cols [64*j]
    g_sb = sb.tile([2 * P, D], f32)
    gT_sb = sb.tile([128, 128], f32r)         # ktile dh at cols [64*dh]
    s_sb = sb.tile([2 * P, D], f32r)
    vodd_sb = sb.tile([128, 512], f32r)       # block blk at cols [256*blk]
    ss0_sb = sb.tile([128, 512], f32r)
    ss1_sb = sb.tile([128, 256], f32r)
    r_sb = sb.tile([128, 1024], f32r)         # retrieved, dh at cols [512*dh]
    sq_sb = sb.tile([128, 1024], f32r)
    rhat_sb = sb.tile([128, 1024], f32r)
    h_sb = sb.tile([128, 2048], f32r)        # mtile m at cols [512*m]
    f_sb = sb.tile([128, 1024], f32r)         # refined
    sq2_sb = sb.tile([128, 1024], f32r)
    y2_sb = sb.tile([128, 1024], f32r)
    out_sb = sb.tile([128, 1024], f32)       # token tile tt at cols [256*tt]

    var1_sb = sb.tile([1, T], f32)
    rstd1_sb = sb.tile([1, T], f32r)
    q1_sb = sb.tile([1, T], f32r)
    m2_sb = sb.tile([1, T], f32)
    stdc_sb = sb.tile([128, 4], f32)
    rstdc_sb = sb.tile([128, 4], f32)
    var2_sb = sb.tile([1, T], f32)
    rstd2_sb = sb.tile([1, T], f32r)
    q2_sb = sb.tile([1, T], f32r)
    m2b_sb = sb.tile([1, T], f32)
    stdc2_sb = sb.tile([128, 4], f32)
    rstdc2_sb = sb.tile([128, 4], f32)

    dma(xc_sb[:], xc.rearrange("(t p) d -> p t d", p=128))
    for kt in range(2):
        dma(xpeT_sb[:, 768 * kt: 768 * kt + 768],
            xpe[:, 128 * kt: 128 * kt + 128], transpose=True)
    dma(xpo_sb[:], xpo.rearrange("(j p) d -> p j d", p=128))

    mm = nc.tensor.matmul
    act = nc.scalar.activation
    tt_ = nc.vector.tensor_tensor
    tcp = nc.vector.tensor_copy

    # ---------------- x^T (own) : 8 PE transposes ----------------
    for tt in range(4):
        for dh in range(2):
            p = pt.tile([128, 128], f32, tag="ptr")
            nc.tensor.transpose(p[:], xc_sb[:, 256 * tt + 128 * dh:
                                            256 * tt + 128 * dh + 128],
                                eyef_sb[:])
            tcp(xT_sb[:, 512 * dh + 128 * tt: 512 * dh + 128 * tt + 128], p[:])

    # ---------------- own phases -> Q ----------------
    ph_ps = pa.tile([P, T], f32, tag="pa")
    for kt in range(2):
        mm(ph_ps[:], _r(kw_sb[:, P * kt: P * kt + P]),
           _r(xT_sb[:, 512 * kt: 512 * kt + 512]),
           start=(kt == 0), stop=(kt == 1))
    act(t_sb[:], ph_ps[:], AF.Tanh, bias=kbc_sb[:])
    act(ta_sb[:], t_sb[:], AF.Abs, bias=zerop_sb[:])
    nc.vector.tensor_scalar(out=ta_sb[:], in0=ta_sb[:], scalar1=-1.0,
                            scalar2=0.5, op0=OP.mult, op1=OP.add)
    act(qb_sb[0:P, :], ta_sb[:], AF.Sin, bias=zerop_sb[:], scale=PI)
    act(qb_sb[P:2 * P, :], t_sb[:], AF.Sin, bias=zerop_sb[:], scale=PI)

    # ---------------- prefix phases -> Kpre ----------------
    pp1 = pa.tile([P, 512], f32, tag="pa")
    pp2 = pa.tile([P, 256], f32, tag="pa")
    for kt in range(2):
        mm(pp1[:], kwb_sb[:, P * kt: P * kt + P],
           xpeT_sb[:, 768 * kt: 768 * kt + 512],
           start=(kt == 0), stop=(kt == 1))
    for kt in range(2):
        mm(pp2[:], kwb_sb[:, P * kt: P * kt + P],
           xpeT_sb[:, 768 * kt + 512: 768 * kt + 768],
           start=(kt == 0), stop=(kt == 1))
    act(tpre_sb[:, 0:512], pp1[:], AF.Tanh, bias=kbc_sb[:])
    act(tpre_sb[:, 512:768], pp2[:], AF.Tanh, bias=kbc_sb[:])
    act(tpa_sb[:], tpre_sb[:], AF.Abs, bias=zerop_sb[:])
    nc.vector.tensor_scalar(out=tpa_sb[:], in0=tpa_sb[:], scalar1=-1.0,
                            scalar2=0.5, op0=OP.mult, op1=OP.add)
    act(kpre_sb[0:P, :], tpa_sb[:], AF.Sin, bias=zerop_sb[:], scale=PI)
    act(kpre_sb[P:2 * P, :], tpre_sb[:], AF.Sin, bias=zerop_sb[:], scale=PI)

    # ---------------- Kpre^T, G, S ----------------
    for j in range(6):
        p = pt.tile([128, 64], bf16, tag="ptr")
        nc.tensor.transpose(p[:], kpre_sb[:, 128 * j: 128 * j + 128],
                            eyeb_sb[0:64, 0:64])
        tcp(kpreT_sb[:, 64 * j: 64 * j + 64], p[:])
    g_ps = pa.tile([2 * P, D], f32, tag="pa")
    for j in range(6):
        mm(g_ps[:], kpreT_sb[:, 64 * j: 64 * j + 64],
           xpo_sb[:, 256 * j: 256 * j + 256],
           start=(j == 0), stop=(j == 5))
    tcp(g_sb[:], g_ps[:])
    for dh in range(2):
        p = pt.tile([128, 64], f32, tag="ptr")
        nc.tensor.transpose(p[:], g_sb[:, 128 * dh: 128 * dh + 128],
                            eyef_sb[0:64, 0:64])
        tcp(gT_sb[:, 64 * dh: 64 * dh + 64], p[:])
    s_ps = pa.tile([2 * P, D], f32, tag="pa")
    for kt in range(2):
        mm(s_ps[:], _r(gT_sb[:, 64 * kt: 64 * kt + 64]),
           _r(vw_sb[:, 256 * kt: 256 * kt + 256]),
           start=(kt == 0), stop=(kt == 1))
    tcp(s_sb[:], s_ps[:])

    # ---------------- V at odd own tokens ----------------
    for blk in range(2):
        vp = pa.tile([128, D], f32, tag="pa")
        for kt in range(2):
            mm(vp[:], _r(xT_sb[:, 512 * kt + 256 * blk + 1:
                               512 * kt + 256 * blk + 256: 2]),
               _r(vw_sb[:, 256 * kt: 256 * kt + 256]),
               start=(kt == 0), stop=(kt == 1))
        tcp(vodd_sb[:, 256 * blk: 256 * blk + 256], vp[:])

    # ---------------- scores (odd tk only) + causal mask ----------------
    sc0 = pb.tile([128, 512], f32, tag="pb")
    mm(sc0[:], _r(qb_sb[:, 0:255:2]), _r(qb_sb[:]), start=True, stop=True)
    tt_(ss0_sb[:, 0:256], sc0[:, 0:256], m0_sb[:], OP.mult)
    tcp(ss0_sb[:, 256:512], sc0[:, 256:512])
    sc1 = pa.tile([128, 256], f32, tag="pa")
    mm(sc1[:], _r(qb_sb[:, 256:511:2]), _r(qb_sb[:, 256:512]),
       start=True, stop=True)
    tt_(ss1_sb[:], sc1[:], m0_sb[:], OP.mult)

    # ---------------- retrieved^T = V^T s + S^T Q ----------------
    retr = []
    for dh in range(2):
        rp = pb.tile([128, 512], f32, tag="pb")
        mm(rp[:], _r(s_sb[:, 128 * dh: 128 * dh + 128]), _r(qb_sb[:]),
           start=True, stop=False)
        mm(rp[:], _r(vodd_sb[:, 128 * dh: 128 * dh + 128]), _r(ss0_sb[:]),
           start=False, stop=False, skip_group_check=True)
        mm(rp[:, 256:512], _r(vodd_sb[:, 256 + 128 * dh: 256 + 128 * dh + 128]),
           _r(ss1_sb[:]), start=False, stop=True, skip_group_check=True)
        retr.append(rp)

    # ---------------- LN1 (folded) ----------------
    def layer_norm(src_ps, src_sb, sqt, stp, msqp, var_sb, m2v_sb, stdc, rstdc,
                   rstd_sb, q_sb, badd):
        # src_ps: 2 psum tiles [128,512] (or None -> read src_sb);
        # copies to src_sb (+ optional per-partition bias), squares, stats,
        # rstd row + broadcast; returns rstdb psum tile [128,512].
        for dh in range(2):
            sl = slice(512 * dh, 512 * dh + 512)
            if src_ps is not None:
                if badd is None:
                    tcp(src_sb[:, sl], src_ps[dh][:])
                    act(sqt[:, sl], src_ps[dh][:], AF.Square, bias=zero128_sb[:])
                else:
                    nc.vector.tensor_scalar(
                        out=src_sb[:, sl], in0=src_ps[dh][:],
                        scalar1=badd[:, dh: dh + 1], scalar2=None, op0=OP.add)
                    act(sqt[:, sl], src_sb[:, sl], AF.Square, bias=zero128_sb[:])
            else:
                act(sqt[:, sl], src_sb[:, sl], AF.Square, bias=zero128_sb[:])
        for kt in range(2):
            mm(stp[0:1, :], _r(invd_sb[:, 0:1]),
               _r(src_sb[:, 512 * kt: 512 * kt + 512]),
               start=(kt == 0), stop=(kt == 1))
        for kt in range(2):
            mm(msqp[0:1, :], _r(invd_sb[:, 0:1]),
               _r(sqt[:, 512 * kt: 512 * kt + 512]),
               start=(kt == 0), stop=(kt == 1))
        act(m2v_sb[:], stp[0:1, :], AF.Square, bias=zero1_sb[:])
        tt_(var_sb[:], msqp[0:1, :], m2v_sb[:], OP.subtract)
        vc = pt.tile([128, 4], f32, tag="ptr")
        for j in range(4):
            nc.tensor.transpose(vc[:, j: j + 1],
                                var_sb[0:1, 128 * j: 128 * j + 128],
                                eyef_sb[0:1, 0:1])
        # rstd = rsqrt(var + eps): magic-seed Newton, all DVE, no ACT table
        i32 = mybir.dt.int32
        nc.vector.tensor_scalar(out=stdc[:], in0=vc[:], scalar1=1e-5,
                                scalar2=None, op0=OP.add)
        nc.vector.tensor_scalar(out=rstdc[:].bitcast(i32),
                                in0=stdc[:].bitcast(i32), scalar1=1,
                                scalar2=None, op0=OP.logical_shift_right)
        nc.vector.tensor_scalar(out=rstdc[:].bitcast(i32),
                                in0=rstdc[:].bitcast(i32), scalar1=-1,
                                scalar2=0x5F3759DF, op0=OP.mult, op1=OP.add)
        nwt = pa.tile([128, 4], f32, tag="pa2", bufs=1)
        for _ in range(3):
            tt_(nwt[:], rstdc[:], rstdc[:], OP.mult)
            tt_(nwt[:], nwt[:], stdc[:], OP.mult)
            nc.vector.tensor_scalar(out=nwt[:], in0=nwt[:], scalar1=-0.5,
                                    scalar2=1.5, op0=OP.mult, op1=OP.add)
            tt_(rstdc[:], rstdc[:], nwt[:], OP.mult)
        rr = pa.tile([1, T], f32, tag="pa")
        for j in range(4):
            nc.tensor.transpose(rr[0:1, 128 * j: 128 * j + 128],
                                rstdc[:, j: j + 1], eyef_sb[:])
        tcp(rstd_sb[:], rr[:])
        tt_(q_sb[:], stp[0:1, :], rstd_sb[:], OP.mult)
        rb = pb.tile([128, 512], f32, tag="pb")
        mm(rb[:], _r(ones_sb[:]), _r(rstd_sb[:]), start=True, stop=True)
        return rb

    st1 = pa.tile([1, T], f32, tag="pa")
    ms1 = pa.tile([1, T], f32, tag="pa2", bufs=1)
    rb1 = layer_norm(retr, r_sb, sq_sb, st1, ms1, var1_sb, m2_sb, stdc_sb,
                     rstdc_sb, rstd1_sb, q1_sb, None)
    for dh in range(2):
        sl = slice(512 * dh, 512 * dh + 512)
        tt_(rhat_sb[:, sl], r_sb[:, sl], rb1[:], OP.mult)

    # ---------------- W1 + rank-1 mean fix + GELU ----------------
    for m in range(4):
        hp = pb.tile([128, 512], f32, tag="pb")
        for kt in range(2):
            mm(hp[:], w1g_sb[:, 512 * kt + 128 * m: 512 * kt + 128 * m + 128],
               rhat_sb[:, 512 * kt: 512 * kt + 512],
               start=(kt == 0), stop=False)
        mm(hp[:], c1n_sb[0:1, 128 * m: 128 * m + 128], q1_sb[:],
           start=False, stop=True, skip_group_check=True)
        act(h_sb[:, 512 * m: 512 * m + 512], hp[:], AF.Gelu,
            bias=cb1_sb[:, m: m + 1])

    # ---------------- W2 -> refined ----------------
    ref = []
    for dh in range(2):
        fp = pb.tile([128, 512], f32, tag="pb")
        for kt in range(4):
            mm(fp[:], w2_sb[:, 256 * kt + 128 * dh: 256 * kt + 128 * dh + 128],
               h_sb[:, 512 * kt: 512 * kt + 512],
               start=(kt == 0), stop=(kt == 3))
        ref.append(fp)

    # ---------------- LN2 (folded) ----------------
    st2 = pa.tile([1, T], f32, tag="pa")
    ms2 = pa.tile([1, T], f32, tag="pa2", bufs=1)
    rb2 = layer_norm(ref, f_sb, sq2_sb, st2, ms2, var2_sb, m2b_sb, stdc2_sb,
                     rstdc2_sb, rstd2_sb, q2_sb, b2c_sb)
    for dh in range(2):
        sl = slice(512 * dh, 512 * dh + 512)
        tt_(y2_sb[:, sl], f_sb[:, sl], rb2[:], OP.mult)

    # ---------------- Wo (token-major) + rank-1 + residual ----------------
    for tm in range(4):
        op = pa.tile([128, D], f32, tag="pa")
        for dh in range(2):
            mm(op[:], y2_sb[:, 512 * dh + 128 * tm: 512 * dh + 128 * tm + 128],
               wog_sb[:, 256 * dh: 256 * dh + 256],
               start=(dh == 0), stop=False)
        mm(op[:], q2_sb[0:1, 128 * tm: 128 * tm + 128], c1on_sb[:],
           start=False, stop=True, skip_group_check=True)
        tt_(out_sb[:, 256 * tm: 256 * tm + 256], op[:],
            xc_sb[:, 256 * tm: 256 * tm + 256], OP.add)
        dma(outc[128 * tm: 128 * tm + 128, :],
            out_sb[:, 256 * tm: 256 * tm + 256])



    pb.release()
    pa.release()
    pt.release()
    sb.release()


_CACHE = {}


def _get_nc():
    if "nc" not in _CACHE:
        _CACHE["nc"] = _build()
    return _CACHE["nc"]


def _bf(a):
    return np.asarray(a, np.float32).astype(ml_dtypes.bfloat16)


def kernel(**inputs):
    x = np.asarray(inputs["x"], np.float32)
    key_W = np.asarray(inputs["key_W"], np.float32)
    key_b = np.asarray(inputs["key_b"], np.float32)
    val_W = np.asarray(inputs["val_W"], np.float32)
    val_b = np.asarray(inputs["val_b"], np.float32)
    ln1_g = np.asarray(inputs["ln1_g"], np.float32)
    ln1_b = np.asarray(inputs["ln1_b"], np.float32)
    W1 = np.asarray(inputs["W1"], np.float32)
    b1 = np.asarray(inputs["b1"], np.float32)
    W2 = np.asarray(inputs["W2"], np.float32)
    b2 = np.asarray(inputs["b2"], np.float32)
    ln2_g = np.asarray(inputs["ln2_g"], np.float32)
    ln2_b = np.asarray(inputs["ln2_b"], np.float32)
    Wo = np.asarray(inputs["Wo"], np.float32)
    bo = np.asarray(inputs["bo"], np.float32)

    # these are identically zero for this module; the kernel folds them out
    assert np.allclose(val_b, 0.0), "nonzero val_b unsupported"
    assert np.allclose(bo + ln2_b @ Wo, 0.0), "nonzero output bias unsupported"

    w1g = ln1_g[:, None] * W1
    wog = ln2_g[:, None] * Wo
    shared = {
        "kw": key_W, "kwb": _bf(key_W), "kbc": key_b.reshape(P, 1),
        "vw": val_W,
        "w1g": w1g, "c1n": -w1g.sum(0, keepdims=True),
        "cb1": (b1 + ln1_b @ W1).reshape(H, 1),
        "w2": W2, "b2c": b2.reshape(D, 1),
        "wog": wog, "c1on": -wog.sum(0, keepdims=True),
        "m0": (np.arange(1, 256, 2)[:, None] <=
               np.arange(256)[None, :]).astype(np.float32),
        "eyef": np.eye(128, dtype=np.float32),
        "onesr": np.ones((1, 128), np.float32),
        "invd": np.full((D, 1), 1.0 / D, np.float32)[:128],
        "eyeb": _bf(np.eye(128)),
    }
    in_maps = []
    for i in range(N_CORES):
        b, c = divmod(i, 4)
        l0 = c * T
        npairs = l0 // 2
        xpe = np.zeros((PRE, D), np.float32)
        xpo = np.zeros((PRE, D), np.float32)
        if npairs:
            xpe[:npairs] = x[b, 0:l0 - 1:2]
            xpo[:npairs] = x[b, 1:l0:2]
        in_maps.append({
            "xc": np.ascontiguousarray(x[b, l0:l0 + T]),
            "xpe": _bf(xpe), "xpo": _bf(xpo), **shared,
        })

    nc = _get_nc()
    res = run_bass_kernel_spmd(nc, in_maps, core_ids=list(range(N_CORES)),
                               **_CACHE.get("run_kwargs", {}))
    _CACHE["last_result"] = res
    out = np.empty((B, L, D), np.float32)
    for i in range(N_CORES):
        b, c = divmod(i, 4)
        out[b, c * T:(c + 1) * T] = res.results[i]["outc"]
    return out


# revision 22
# speedup vs baseline: 1.0649x; 1.0649x over previous
"""Trainium2 Bass kernel for nn_HardcodedKVMemoryBlock (8 NeuronCores).

Sharding: core i handles batch b=i//4, sequence chunk c=i%4 (512 tokens).
The (B,L,P,D) cumsum is restructured as causal linear attention:
    retrieved = tril(Q K^T) @ V + Q @ S_prefix
with Q=[cos,sin] phasors (L x 64), V = values at odd positions, and the
cross-chunk carry S_prefix = (K_even^T @ x_odd) @ val_W computed
redundantly per core from a zero-padded prefix (no collectives).
The 1/sqrt(valid*P) normalization cancels inside LayerNorm1 (scale
invariance); ln gains are folded into W1/Wo and means are applied as
rank-1 PE updates, so LN costs no extra full-width element-wise passes
beyond one multiply by the broadcast rstd.
"""

import math
import numpy as np
import ml_dtypes

import concourse.bass as bass
import concourse.tile as tile
from concourse import bacc, mybir
from concourse.bass_utils import run_bass_kernel_spmd

PI = math.pi
B, L, D, P = 2, 2048, 256, 32
T = 512          # own tokens per core
H = 512          # MLP hidden
PRE = 768        # padded prefix pair count (max prefix 1536 tokens / 2)
N_CORES = 8

f32 = mybir.dt.float32
f32r = mybir.dt.float32r
bf16 = mybir.dt.bfloat16
AF = mybir.ActivationFunctionType
OP = mybir.AluOpType


def _r(ap):
    return ap.bitcast(f32r)


def _build():
    nc = bacc.Bacc("TRN2", target_bir_lowering=False, debug=False,
                   num_devices=N_CORES)

    def din(name, shape, dt):
        return nc.dram_tensor(name, shape, dt, kind="ExternalInput").ap()

    xc = din("xc", [T, D], f32)
    xpe = din("xpe", [PRE, D], bf16)
    xpo = din("xpo", [PRE, D], bf16)
    kw = din("kw", [D, P], f32r)
    kwb = din("kwb", [D, P], bf16)
    kbc = din("kbc", [P, 1], f32)
    vw = din("vw", [D, D], f32r)
    w1g = din("w1g", [D, H], f32r)
    c1n = din("c1n", [1, H], f32r)
    cb1 = din("cb1", [H, 1], f32)
    w2 = din("w2", [H, D], f32r)
    b2c = din("b2c", [D, 1], f32)
    wog = din("wog", [D, D], f32r)
    c1on = din("c1on", [1, D], f32r)
    m0 = din("m0", [128, 256], f32)
    onesr = din("onesr", [1, 128], f32r)
    invd = din("invd", [128, 1], f32r)
    eyef = din("eyef", [128, 128], f32)
    eyeb = din("eyeb", [128, 128], bf16)
    outc = nc.dram_tensor("outc", [T, D], f32, kind="ExternalOutput").ap()

    with tile.TileContext(nc) as tc:
        _emit(tc, locals())
    nc.compile()
    return nc


def _emit(tc, io):
    nc = tc.nc
    xc, xpe, xpo = io["xc"], io["xpe"], io["xpo"]
    outc = io["outc"]

    sb = tc.alloc_tile_pool(name="sb", bufs=1)
    pt = tc.alloc_tile_pool(name="pt", bufs=2, space="PSUM")    # transposes
    pa = tc.alloc_tile_pool(name="pa", bufs=2, space="PSUM")    # narrow tiles
    pb = tc.alloc_tile_pool(name="pb", bufs=3, space="PSUM")    # [128,512]

    # ---------------- constant / weight tiles ----------------
    kw_sb = sb.tile([128, 2 * P], f32r)       # ktile kt at cols [P*kt]
    kwb_sb = sb.tile([128, 2 * P], bf16)
    kbc_sb = sb.tile([P, 1], f32)
    vw_sb = sb.tile([128, 512], f32r)         # ktile kt at cols [256*kt]
    w1g_sb = sb.tile([128, 1024], f32r)      # ktile kt at cols [512*kt]
    c1n_sb = sb.tile([1, H], f32r)
    cb1_sb = sb.tile([128, 4], f32)          # mtile m at col m
    w2_sb = sb.tile([128, 1024], f32r)       # ktile kt at cols [256*kt]
    b2c_sb = sb.tile([128, 2], f32)
    wog_sb = sb.tile([128, 512], f32r)       # ktile dh at cols [256*dh]
    c1on_sb = sb.tile([1, D], f32r)
    m0_sb = sb.tile([128, 256], f32)
    eyef_sb = sb.tile([128, 128], f32)
    eyeb_sb = sb.tile([128, 128], bf16)
    ones_sb = sb.tile([1, 128], f32r)
    invd_sb = sb.tile([128, 1], f32r)
    epsb_sb = sb.tile([128, 1], f32)
    halfpi_sb = sb.tile([P, 1], f32)
    zerop_sb = sb.tile([P, 1], f32)
    zero128_sb = sb.tile([128, 1], f32)
    zero1_sb = sb.tile([1, 1], f32)

    dma = nc.sync.dma_start
    dmaw = nc.gpsimd.dma_start
    nc.vector.memset(epsb_sb[:], 1e-5)
    nc.vector.memset(halfpi_sb[:], PI / 2)
    nc.vector.memset(zerop_sb[:], 0.0)
    nc.vector.memset(zero128_sb[:], 0.0)
    nc.vector.memset(zero1_sb[:], 0.0)

    # ---------------- data tiles ----------------
    xc_sb = sb.tile([128, 1024], f32)        # token tile tt at cols [256*tt]
    xT_sb = sb.tile([128, 1024], f32r)        # d-half dh at cols [512*dh]
    xpe_sb = sb.tile([128, 1536], bf16)      # block j at cols [256*j]
    xpo_sb = sb.tile([128, 1536], bf16)
    xpeT_sb = sb.tile([128, 1536], bf16)     # ktile kt at cols [768*kt]
    t_sb = sb.tile([P, T], f32)
    ta_sb = sb.tile([P, T], f32)
    qb_sb = sb.tile([2 * P, T], f32r)         # rows 0:32 cos, 32:64 sin
    tpre_sb = sb.tile([P, PRE], f32)
    tpa_sb = sb.tile([P, PRE], f32)
    kpre_sb = sb.tile([2 * P, PRE], bf16)
    kpreT_sb = sb.tile([128, 6 * 64], bf16)  # block j at cols [64*j]
    g_sb = sb.tile([2 * P, D], f32)
    gT_sb = sb.tile([128, 128], f32r)         # ktile dh at cols [64*dh]
    s_sb = sb.tile([2 * P, D], f32r)
    vodd_sb = sb.tile([128, 512], f32r)       # block blk at cols [256*blk]
    ss0_sb = sb.tile([128, 512], f32r)
    ss1_sb = sb.tile([128, 256], f32r)
    r_sb = sb.tile([128, 1024], f32r)         # retrieved, dh at cols [512*dh]
    sq_sb = sb.tile([128, 1024], f32r)
    rhat_sb = sb.tile([128, 1024], f32r)
    h_sb = sb.tile([128, 2048], f32r)        # mtile m at cols [512*m]
    f_sb = sb.tile([128, 1024], f32r)         # refined
    sq2_sb = sb.tile([128, 1024], f32r)
    y2_sb = sb.tile([128, 1024], f32r)
    out_sb = sb.tile([128, 1024], f32)       # token tile tt at cols [256*tt]

    var1_sb = sb.tile([1, T], f32)
    rstd1_sb = sb.tile([1, T], f32r)
    q1_sb = sb.tile([1, T], f32r)
    m2_sb = sb.tile([1, T], f32)
    stdc_sb = sb.tile([128, 4], f32)
    rstdc_sb = sb.tile([128, 4], f32)
    var2_sb = sb.tile([1, T], f32)
    rstd2_sb = sb.tile([1, T], f32r)
    q2_sb = sb.tile([1, T], f32r)
    m2b_sb = sb.tile([1, T], f32)
    stdc2_sb = sb.tile([128, 4], f32)
    rstdc2_sb = sb.tile([128, 4], f32)


    dma(xc_sb[:], xc.rearrange("(t p) d -> p t d", p=128))
    dma(xpe_sb[:], xpe.rearrange("(j p) d -> p j d", p=128))
    dma(xpo_sb[:], xpo.rearrange("(j p) d -> p j d", p=128))
    dma(eyef_sb[:], io["eyef"])
    dma(kw_sb[:], io["kw"].rearrange("(k p) q -> p k q", p=128))
    dma(kwb_sb[:], io["kwb"].rearrange("(k p) q -> p k q", p=128))
    dma(kbc_sb[:], io["kbc"])
    dma(eyeb_sb[:], io["eyeb"])
    dma(ones_sb[:], io["onesr"])
    dma(invd_sb[:], io["invd"])
    dmaw(vw_sb[:], io["vw"].rearrange("(k p) q -> p k q", p=128))
    dmaw(w1g_sb[:], io["w1g"].rearrange("(k p) q -> p k q", p=128))
    dmaw(c1n_sb[:], io["c1n"])
    dmaw(cb1_sb[:], io["cb1"].rearrange("(m p) o -> p m o", p=128))
    dmaw(w2_sb[:], io["w2"].rearrange("(k p) q -> p k q", p=128))
    dmaw(b2c_sb[:], io["b2c"].rearrange("(m p) o -> p m o", p=128))
    dmaw(wog_sb[:], io["wog"].rearrange("(k p) q -> p k q", p=128))
    dmaw(c1on_sb[:], io["c1on"])
    dmaw(m0_sb[:], io["m0"])

    mm = nc.tensor.matmul
    act = nc.scalar.activation
    tt_ = nc.vector.tensor_tensor
    tcp = nc.vector.tensor_copy

    # ---------------- x^T (own) : 8 PE transposes ----------------
    for tt in range(4):
        for dh in range(2):
            p = pt.tile([128, 128], f32, tag="ptr")
            nc.tensor.transpose(p[:], xc_sb[:, 256 * tt + 128 * dh:
                                            256 * tt + 128 * dh + 128],
                                eyef_sb[:])
            tcp(xT_sb[:, 512 * dh + 128 * tt: 512 * dh + 128 * tt + 128], p[:])

    # ---------------- prefix x_even^T : 12 PE transposes ----------------
    for j in range(6):
        for kt in range(2):
            p = pt.tile([128, 128], bf16, tag="ptr")
            nc.tensor.transpose(p[:], xpe_sb[:, 256 * j + 128 * kt:
                                             256 * j + 128 * kt + 128],
                                eyeb_sb[:])
            tcp(xpeT_sb[:, 768 * kt + 128 * j: 768 * kt + 128 * j + 128],
                p[:])

    # ---------------- own phases -> Q ----------------
    ph_ps = pa.tile([P, T], f32, tag="pa")
    for kt in range(2):
        mm(ph_ps[:], _r(kw_sb[:, P * kt: P * kt + P]),
           _r(xT_sb[:, 512 * kt: 512 * kt + 512]),
           start=(kt == 0), stop=(kt == 1))
    act(t_sb[:], ph_ps[:], AF.Tanh, bias=kbc_sb[:])
    ab_i = act(ta_sb[:], t_sb[:], AF.Abs, bias=zerop_sb[:])
    nc.vector.tensor_scalar(out=ta_sb[:], in0=ta_sb[:], scalar1=-1.0,
                            scalar2=0.5, op0=OP.mult, op1=OP.add)
    qs_i1 = act(qb_sb[0:P, :], ta_sb[:], AF.Sin, bias=zerop_sb[:], scale=PI)
    qs_i2 = act(qb_sb[P:2 * P, :], t_sb[:], AF.Sin, bias=zerop_sb[:], scale=PI)

    # ---------------- prefix phases -> Kpre ----------------
    pp1 = pa.tile([P, 512], f32, tag="pa")
    pp2 = pa.tile([P, 256], f32, tag="pa")
    for kt in range(2):
        mm(pp1[:], kwb_sb[:, P * kt: P * kt + P],
           xpeT_sb[:, 768 * kt: 768 * kt + 512],
           start=(kt == 0), stop=(kt == 1))
    for kt in range(2):
        mm(pp2[:], kwb_sb[:, P * kt: P * kt + P],
           xpeT_sb[:, 768 * kt + 512: 768 * kt + 768],
           start=(kt == 0), stop=(kt == 1))
    tp_i1 = act(tpre_sb[:, 0:512], pp1[:], AF.Tanh, bias=kbc_sb[:])
    tp_i2 = act(tpre_sb[:, 512:768], pp2[:], AF.Tanh, bias=kbc_sb[:])
    # keep both Tanh ops adjacent on ACT before any Sin (table grouping)
    from concourse.tile import add_dep_helper
    for si in (ab_i, qs_i1, qs_i2):
        for ti in (tp_i1, tp_i2):
            add_dep_helper(si.ins, ti.ins, sync=False,
                           reason="group tanh before sin for act tables")
    act(tpa_sb[:], tpre_sb[:], AF.Abs, bias=zerop_sb[:])
    nc.vector.tensor_scalar(out=tpa_sb[:], in0=tpa_sb[:], scalar1=-1.0,
                            scalar2=0.5, op0=OP.mult, op1=OP.add)
    act(kpre_sb[0:P, :], tpa_sb[:], AF.Sin, bias=zerop_sb[:], scale=PI)
    act(kpre_sb[P:2 * P, :], tpre_sb[:], AF.Sin, bias=zerop_sb[:], scale=PI)

    # ---------------- Kpre^T, G, S ----------------
    for j in range(6):
        p = pt.tile([128, 64], bf16, tag="ptr")
        nc.tensor.transpose(p[:], kpre_sb[:, 128 * j: 128 * j + 128],
                            eyeb_sb[0:64, 0:64])
        tcp(kpreT_sb[:, 64 * j: 64 * j + 64], p[:])
    g_ps = pa.tile([2 * P, D], f32, tag="pa")
    for j in range(6):
        mm(g_ps[:], kpreT_sb[:, 64 * j: 64 * j + 64],
           xpo_sb[:, 256 * j: 256 * j + 256],
           start=(j == 0), stop=(j == 5))
    tcp(g_sb[:], g_ps[:])
    for dh in range(2):
        p = pt.tile([128, 64], f32, tag="ptr")
        nc.tensor.transpose(p[:], g_sb[:, 128 * dh: 128 * dh + 128],
                            eyef_sb[0:64, 0:64])
        tcp(gT_sb[:, 64 * dh: 64 * dh + 64], p[:])
    s_ps = pa.tile([2 * P, D], f32, tag="pa")
    for kt in range(2):
        mm(s_ps[:], _r(gT_sb[:, 64 * kt: 64 * kt + 64]),
           _r(vw_sb[:, 256 * kt: 256 * kt + 256]),
           start=(kt == 0), stop=(kt == 1))
    tcp(s_sb[:], s_ps[:])

    # ---------------- V at odd own tokens ----------------
    for blk in range(2):
        vp = pa.tile([128, D], f32, tag="pa")
        for kt in range(2):
            mm(vp[:], _r(xT_sb[:, 512 * kt + 256 * blk + 1:
                               512 * kt + 256 * blk + 256: 2]),
               _r(vw_sb[:, 256 * kt: 256 * kt + 256]),
               start=(kt == 0), stop=(kt == 1))
        tcp(vodd_sb[:, 256 * blk: 256 * blk + 256], vp[:])

    # ---------------- scores (odd tk only) + causal mask ----------------
    sc0 = pb.tile([128, 512], f32, tag="pb")
    mm(sc0[:], _r(qb_sb[:, 0:255:2]), _r(qb_sb[:]), start=True, stop=True)
    tt_(ss0_sb[:, 0:256], sc0[:, 0:256], m0_sb[:], OP.mult)
    tcp(ss0_sb[:, 256:512], sc0[:, 256:512])
    sc1 = pa.tile([128, 256], f32, tag="pa")
    mm(sc1[:], _r(qb_sb[:, 256:511:2]), _r(qb_sb[:, 256:512]),
       start=True, stop=True)
    tt_(ss1_sb[:], sc1[:], m0_sb[:], OP.mult)

    # ---------------- retrieved^T = V^T s + S^T Q ----------------
    retr = []
    for dh in range(2):
        rp = pb.tile([128, 512], f32, tag="pb")
        mm(rp[:], _r(s_sb[:, 128 * dh: 128 * dh + 128]), _r(qb_sb[:]),
           start=True, stop=False)
        mm(rp[:], _r(vodd_sb[:, 128 * dh: 128 * dh + 128]), _r(ss0_sb[:]),
           start=False, stop=False, skip_group_check=True)
        mm(rp[:, 256:512], _r(vodd_sb[:, 256 + 128 * dh: 256 + 128 * dh + 128]),
           _r(ss1_sb[:]), start=False, stop=True, skip_group_check=True)
        retr.append(rp)

    # ---------------- LN1 (folded) ----------------
    def layer_norm(src_ps, src_sb, sqt, stp, msqp, var_sb, m2v_sb, stdc, rstdc,
                   rstd_sb, q_sb, badd):
        # src_ps: 2 psum tiles [128,512] (or None -> read src_sb);
        # copies to src_sb (+ optional per-partition bias), squares, stats,
        # rstd row + broadcast; returns rstdb psum tile [128,512].
        for dh in range(2):
            sl = slice(512 * dh, 512 * dh + 512)
            if src_ps is not None:
                if badd is None:
                    tcp(src_sb[:, sl], src_ps[dh][:])
                    act(sqt[:, sl], src_ps[dh][:], AF.Square, bias=zero128_sb[:])
                else:
                    nc.vector.tensor_scalar(
                        out=src_sb[:, sl], in0=src_ps[dh][:],
                        scalar1=badd[:, dh: dh + 1], scalar2=None, op0=OP.add)
                    act(sqt[:, sl], src_sb[:, sl], AF.Square, bias=zero128_sb[:])
            else:
                act(sqt[:, sl], src_sb[:, sl], AF.Square, bias=zero128_sb[:])
        for kt in range(2):
            mm(stp[0:1, :], _r(invd_sb[:, 0:1]),
               _r(src_sb[:, 512 * kt: 512 * kt + 512]),
               start=(kt == 0), stop=(kt == 1))
        for kt in range(2):
            mm(msqp[0:1, :], _r(invd_sb[:, 0:1]),
               _r(sqt[:, 512 * kt: 512 * kt + 512]),
               start=(kt == 0), stop=(kt == 1))
        act(m2v_sb[:], stp[0:1, :], AF.Square, bias=zero1_sb[:])
        tt_(var_sb[:], msqp[0:1, :], m2v_sb[:], OP.subtract)
        vc = pt.tile([128, 4], f32, tag="ptr")
        for j in range(4):
            nc.tensor.transpose(vc[:, j: j + 1],
                                var_sb[0:1, 128 * j: 128 * j + 128],
                                eyef_sb[0:1, 0:1])
        # rstd = rsqrt(var + eps): magic-seed Newton, all DVE, no ACT table
        i32 = mybir.dt.int32
        nc.vector.tensor_scalar(out=stdc[:], in0=vc[:], scalar1=1e-5,
                                scalar2=None, op0=OP.add)
        nc.vector.tensor_scalar(out=rstdc[:].bitcast(i32),
                                in0=stdc[:].bitcast(i32), scalar1=1,
                                scalar2=None, op0=OP.logical_shift_right)
        nc.vector.tensor_scalar(out=rstdc[:].bitcast(i32),
                                in0=rstdc[:].bitcast(i32), scalar1=-1,
                                scalar2=0x5F3759DF, op0=OP.mult, op1=OP.add)
        nwt = pa.tile([128, 4], f32, tag="pa2", bufs=1)
        for _ in range(3):
            tt_(nwt[:], rstdc[:], rstdc[:], OP.mult)
            tt_(nwt[:], nwt[:], stdc[:], OP.mult)
            nc.vector.tensor_scalar(out=nwt[:], in0=nwt[:], scalar1=-0.5,
                                    scalar2=1.5, op0=OP.mult, op1=OP.add)
            tt_(rstdc[:], rstdc[:], nwt[:], OP.mult)
        rr = pa.tile([1, T], f32, tag="pa")
        for j in range(4):
            nc.tensor.transpose(rr[0:1, 128 * j: 128 * j + 128],
                                rstdc[:, j: j + 1], eyef_sb[:])
        tcp(rstd_sb[:], rr[:])
        tt_(q_sb[:], stp[0:1, :], rstd_sb[:], OP.mult)
        rb = pb.tile([128, 512], f32, tag="pb")
        mm(rb[:], _r(ones_sb[:]), _r(rstd_sb[:]), start=True, stop=True)
        return rb

    st1 = pa.tile([1, T], f32, tag="pa")
    ms1 = pa.tile([1, T], f32, tag="pa2", bufs=1)
    rb1 = layer_norm(retr, r_sb, sq_sb, st1, ms1, var1_sb, m2_sb, stdc_sb,
                     rstdc_sb, rstd1_sb, q1_sb, None)
    for dh in range(2):
        sl = slice(512 * dh, 512 * dh + 512)
        tt_(rhat_sb[:, sl], r_sb[:, sl], rb1[:], OP.mult)

    # ---------------- W1 + rank-1 mean fix + GELU ----------------
    for m in range(4):
        hp = pb.tile([128, 512], f32, tag="pb")
        for kt in range(2):
            mm(hp[:], w1g_sb[:, 512 * kt + 128 * m: 512 * kt + 128 * m + 128],
               rhat_sb[:, 512 * kt: 512 * kt + 512],
               start=(kt == 0), stop=False)
        mm(hp[:], c1n_sb[0:1, 128 * m: 128 * m + 128], q1_sb[:],
           start=False, stop=True, skip_group_check=True)
        act(h_sb[:, 512 * m: 512 * m + 512], hp[:], AF.Gelu,
            bias=cb1_sb[:, m: m + 1])

    # ---------------- W2 -> refined ----------------
    ref = []
    for dh in range(2):
        fp = pb.tile([128, 512], f32, tag="pb")
        for kt in range(4):
            mm(fp[:], w2_sb[:, 256 * kt + 128 * dh: 256 * kt + 128 * dh + 128],
               h_sb[:, 512 * kt: 512 * kt + 512],
               start=(kt == 0), stop=(kt == 3))
        ref.append(fp)

    # ---------------- LN2 (folded) ----------------
    st2 = pa.tile([1, T], f32, tag="pa")
    ms2 = pa.tile([1, T], f32, tag="pa2", bufs=1)
    rb2 = layer_norm(ref, f_sb, sq2_sb, st2, ms2, var2_sb, m2b_sb, stdc2_sb,
                     rstdc2_sb, rstd2_sb, q2_sb, b2c_sb)
    for dh in range(2):
        sl = slice(512 * dh, 512 * dh + 512)
        tt_(y2_sb[:, sl], f_sb[:, sl], rb2[:], OP.mult)

    # ---------------- Wo (token-major) + rank-1 + residual ----------------
    for tm in range(4):
        op = pa.tile([128, D], f32, tag="pa")
        for dh in range(2):
            mm(op[:], y2_sb[:, 512 * dh + 128 * tm: 512 * dh + 128 * tm + 128],
               wog_sb[:, 256 * dh: 256 * dh + 256],
               start=(dh == 0), stop=False)
        mm(op[:], q2_sb[0:1, 128 * tm: 128 * tm + 128], c1on_sb[:],
           start=False, stop=True, skip_group_check=True)
        tt_(out_sb[:, 256 * tm: 256 * tm + 256], op[:],
            xc_sb[:, 256 * tm: 256 * tm + 256], OP.add)
        dma(outc[128 * tm: 128 * tm + 128, :],
            out_sb[:, 256 * tm: 256 * tm + 256])



    pb.release()
    pa.release()
    pt.release()
    sb.release()


_CACHE = {}


def _get_nc():
    if "nc" not in _CACHE:
        _CACHE["nc"] = _build()
    return _CACHE["nc"]


def _bf(a):
    return np.asarray(a, np.float32).astype(ml_dtypes.bfloat16)


def kernel(**inputs):
    x = np.asarray(inputs["x"], np.float32)
    key_W = np.asarray(inputs["key_W"], np.float32)
    key_b = np.asarray(inputs["key_b"], np.float32)
    val_W = np.asarray(inputs["val_W"], np.float32)
    val_b = np.asarray(inputs["val_b"], np.float32)
    ln1_g = np.asarray(inputs["ln1_g"], np.float32)
    ln1_b = np.asarray(inputs["ln1_b"], np.float32)
    W1 = np.asarray(inputs["W1"], np.float32)
    b1 = np.asarray(inputs["b1"], np.float32)
    W2 = np.asarray(inputs["W2"], np.float32)
    b2 = np.asarray(inputs["b2"], np.float32)
    ln2_g = np.asarray(inputs["ln2_g"], np.float32)
    ln2_b = np.asarray(inputs["ln2_b"], np.float32)
    Wo = np.asarray(inputs["Wo"], np.float32)
    bo = np.asarray(inputs["bo"], np.float32)

    # these are identically zero for this module; the kernel folds them out
    assert np.allclose(val_b, 0.0), "nonzero val_b unsupported"
    assert np.allclose(bo + ln2_b @ Wo, 0.0), "nonzero output bias unsupported"

    w1g = ln1_g[:, None] * W1
    wog = ln2_g[:, None] * Wo
    shared = {
        "kw": key_W, "kwb": _bf(key_W), "kbc": key_b.reshape(P, 1),
        "vw": val_W,
        "w1g": w1g, "c1n": -w1g.sum(0, keepdims=True),
        "cb1": (b1 + ln1_b @ W1).reshape(H, 1),
        "w2": W2, "b2c": b2.reshape(D, 1),
        "wog": wog, "c1on": -wog.sum(0, keepdims=True),
        "m0": (np.arange(1, 256, 2)[:, None] <=
               np.arange(256)[None, :]).astype(np.float32),
        "eyef": np.eye(128, dtype=np.float32),
        "onesr": np.ones((1, 128), np.float32),
        "invd": np.full((D, 1), 1.0 / D, np.float32)[:128],
        "eyeb": _bf(np.eye(128)),
    }
    in_maps = []
    for i in range(N_CORES):
        b, c = divmod(i, 4)
        l0 = c * T
        npairs = l0 // 2
        xpe = np.zeros((PRE, D), np.float32)
        xpo = np.zeros((PRE, D), np.float32)
        if npairs:
            xpe[:npairs] = x[b, 0:l0 - 1:2]
            xpo[:npairs] = x[b, 1:l0:2]
        in_maps.append({
            "xc": np.ascontiguousarray(x[b, l0:l0 + T]),
            "xpe": _bf(xpe), "xpo": _bf(xpo), **shared,
        })

    nc = _get_nc()
    res = run_bass_kernel_spmd(nc, in_maps, core_ids=list(range(N_CORES)),
                               **_CACHE.get("run_kwargs", {}))
    _CACHE["last_result"] = res
    out = np.empty((B, L, D), np.float32)
    for i in range(N_CORES):
        b, c = divmod(i, 4)
        out[b, c * T:(c + 1) * T] = res.results[i]["outc"]
    return out


# revision 23
# speedup vs baseline: 1.1629x; 1.0920x over previous
"""Trainium2 Bass kernel for nn_HardcodedKVMemoryBlock (8 NeuronCores).

Sharding: core i handles batch b=i//4, sequence chunk c=i%4 (512 tokens).
The (B,L,P,D) cumsum is restructured as causal linear attention:
    retrieved = tril(Q K^T) @ V + Q @ S_prefix
with Q=[cos,sin] phasors (L x 64), V = values at odd positions, and the
cross-chunk carry S_prefix = (K_even^T @ x_odd) @ val_W computed
redundantly per core from a zero-padded prefix (no collectives).
The 1/sqrt(valid*P) normalization cancels inside LayerNorm1 (scale
invariance); ln gains are folded into W1/Wo and means are applied as
rank-1 PE updates, so LN costs no extra full-width element-wise passes
beyond one multiply by the broadcast rstd.
"""

import math
import numpy as np
import ml_dtypes

import concourse.bass as bass
import concourse.tile as tile
from concourse import bacc, mybir
from concourse.bass_utils import run_bass_kernel_spmd

PI = math.pi
B, L, D, P = 2, 2048, 256, 32
T = 512          # own tokens per core
H = 512          # MLP hidden
PRE = 768        # padded prefix pair count (max prefix 1536 tokens / 2)
N_CORES = 8

f32 = mybir.dt.float32
f32r = mybir.dt.float32r
bf16 = mybir.dt.bfloat16
AF = mybir.ActivationFunctionType
OP = mybir.AluOpType


def _r(ap):
    return ap.bitcast(f32r)


def _build():
    nc = bacc.Bacc("TRN2", target_bir_lowering=False, debug=False,
                   num_devices=N_CORES)

    def din(name, shape, dt):
        return nc.dram_tensor(name, shape, dt, kind="ExternalInput").ap()

    xc = din("xc", [T, D], f32)
    xpe = din("xpe", [PRE, D], bf16)
    xpo = din("xpo", [PRE, D], bf16)
    kw = din("kw", [D, P], f32r)
    kwb = din("kwb", [D, P], bf16)
    kbc = din("kbc", [P, 1], f32)
    vw = din("vw", [D, D], f32r)
    w1g = din("w1g", [D, H], bf16)
    c1n = din("c1n", [1, H], bf16)
    cb1 = din("cb1", [H, 1], f32)
    w2 = din("w2", [H, D], bf16)
    b2c = din("b2c", [D, 1], f32)
    wog = din("wog", [D, D], bf16)
    c1on = din("c1on", [1, D], bf16)
    m0 = din("m0", [128, 256], f32)
    onesr = din("onesr", [1, 128], f32r)
    invd = din("invd", [128, 1], f32r)
    eyef = din("eyef", [128, 128], f32)
    eyeb = din("eyeb", [128, 128], bf16)
    outc = nc.dram_tensor("outc", [T, D], f32, kind="ExternalOutput").ap()

    with tile.TileContext(nc) as tc:
        _emit(tc, locals())
    nc.compile()
    return nc


def _emit(tc, io):
    nc = tc.nc
    xc, xpe, xpo = io["xc"], io["xpe"], io["xpo"]
    outc = io["outc"]

    sb = tc.alloc_tile_pool(name="sb", bufs=1)
    pt = tc.alloc_tile_pool(name="pt", bufs=2, space="PSUM")    # transposes
    pa = tc.alloc_tile_pool(name="pa", bufs=2, space="PSUM")    # narrow tiles
    pb = tc.alloc_tile_pool(name="pb", bufs=3, space="PSUM")    # [128,512]

    # ---------------- constant / weight tiles ----------------
    kw_sb = sb.tile([128, 2 * P], f32r)       # ktile kt at cols [P*kt]
    kwb_sb = sb.tile([128, 2 * P], bf16)
    kbc_sb = sb.tile([P, 1], f32)
    vw_sb = sb.tile([128, 512], f32r)         # ktile kt at cols [256*kt]
    w1g_sb = sb.tile([128, 1024], bf16)      # ktile kt at cols [512*kt]
    c1n_sb = sb.tile([1, H], bf16)
    cb1_sb = sb.tile([128, 4], f32)          # mtile m at col m
    w2_sb = sb.tile([128, 1024], bf16)       # ktile kt at cols [256*kt]
    b2c_sb = sb.tile([128, 2], f32)
    wog_sb = sb.tile([128, 512], bf16)       # ktile dh at cols [256*dh]
    c1on_sb = sb.tile([1, D], bf16)
    m0_sb = sb.tile([128, 256], f32)
    eyef_sb = sb.tile([128, 128], f32)
    eyeb_sb = sb.tile([128, 128], bf16)
    ones_sb = sb.tile([1, 128], f32r)
    invd_sb = sb.tile([128, 1], f32r)
    epsb_sb = sb.tile([128, 1], f32)
    halfpi_sb = sb.tile([P, 1], f32)
    zerop_sb = sb.tile([P, 1], f32)
    zero128_sb = sb.tile([128, 1], f32)
    zero1_sb = sb.tile([1, 1], f32)

    dma = nc.sync.dma_start
    dmaw = nc.gpsimd.dma_start
    nc.vector.memset(epsb_sb[:], 1e-5)
    nc.vector.memset(halfpi_sb[:], PI / 2)
    nc.vector.memset(zerop_sb[:], 0.0)
    nc.vector.memset(zero128_sb[:], 0.0)
    nc.vector.memset(zero1_sb[:], 0.0)

    # ---------------- data tiles ----------------
    xc_sb = sb.tile([128, 1024], f32)        # token tile tt at cols [256*tt]
    xT_sb = sb.tile([128, 1024], f32r)        # d-half dh at cols [512*dh]
    xpe_sb = sb.tile([128, 1536], bf16)      # block j at cols [256*j]
    xpo_sb = sb.tile([128, 1536], bf16)
    xpeT_sb = sb.tile([128, 1536], bf16)     # ktile kt at cols [768*kt]
    t_sb = sb.tile([P, T], f32)
    ta_sb = sb.tile([P, T], f32)
    qb_sb = sb.tile([2 * P, T], f32r)         # rows 0:32 cos, 32:64 sin
    tpre_sb = sb.tile([P, PRE], f32)
    tpa_sb = sb.tile([P, PRE], f32)
    kpre_sb = sb.tile([2 * P, PRE], bf16)
    kpreT_sb = sb.tile([128, 6 * 64], bf16)  # block j at cols [64*j]
    g_sb = sb.tile([2 * P, D], f32)
    gT_sb = sb.tile([128, 128], f32r)         # ktile dh at cols [64*dh]
    s_sb = sb.tile([2 * P, D], f32r)
    vodd_sb = sb.tile([128, 512], f32r)       # block blk at cols [256*blk]
    ss0_sb = sb.tile([128, 512], f32r)
    ss1_sb = sb.tile([128, 256], f32r)
    r_sb = sb.tile([128, 1024], f32r)         # retrieved, dh at cols [512*dh]
    sq_sb = sb.tile([128, 1024], f32r)
    rhat_sb = sb.tile([128, 1024], bf16)
    h_sb = sb.tile([128, 2048], bf16)        # mtile m at cols [512*m]
    f_sb = sb.tile([128, 1024], f32r)         # refined
    sq2_sb = sb.tile([128, 1024], f32r)
    y2_sb = sb.tile([128, 1024], bf16)
    out_sb = sb.tile([128, 1024], f32)       # token tile tt at cols [256*tt]

    var1_sb = sb.tile([1, T], f32)
    rstd1_sb = sb.tile([1, T], f32r)
    q1_sb = sb.tile([1, T], bf16)
    m2_sb = sb.tile([1, T], f32)
    stdc_sb = sb.tile([128, 4], f32)
    rstdc_sb = sb.tile([128, 4], f32)
    var2_sb = sb.tile([1, T], f32)
    rstd2_sb = sb.tile([1, T], f32r)
    q2_sb = sb.tile([1, T], bf16)
    m2b_sb = sb.tile([1, T], f32)
    stdc2_sb = sb.tile([128, 4], f32)
    rstdc2_sb = sb.tile([128, 4], f32)


    from concourse.tile import add_dep_helper
    d_xc = dma(xc_sb[:], xc.rearrange("(t p) d -> p t d", p=128))
    d_xpe = dma(xpe_sb[:], xpe.rearrange("(j p) d -> p j d", p=128))
    d_xpo = dma(xpo_sb[:], xpo.rearrange("(j p) d -> p j d", p=128))
    dma(eyef_sb[:], io["eyef"])
    dma(kw_sb[:], io["kw"].rearrange("(k p) q -> p k q", p=128))
    dma(kwb_sb[:], io["kwb"].rearrange("(k p) q -> p k q", p=128))
    dma(kbc_sb[:], io["kbc"])
    dma(eyeb_sb[:], io["eyeb"])
    dma(ones_sb[:], io["onesr"])
    dma(invd_sb[:], io["invd"])
    wd = []
    wd.append(dmaw(vw_sb[:], io["vw"].rearrange("(k p) q -> p k q", p=128)))
    wd.append(dmaw(w1g_sb[:], io["w1g"].rearrange("(k p) q -> p k q", p=128)))
    wd.append(dmaw(c1n_sb[:], io["c1n"]))
    wd.append(dmaw(cb1_sb[:], io["cb1"].rearrange("(m p) o -> p m o", p=128)))
    wd.append(dmaw(w2_sb[:], io["w2"].rearrange("(k p) q -> p k q", p=128)))
    wd.append(dmaw(b2c_sb[:], io["b2c"].rearrange("(m p) o -> p m o", p=128)))
    wd.append(dmaw(wog_sb[:], io["wog"].rearrange("(k p) q -> p k q", p=128)))
    wd.append(dmaw(c1on_sb[:], io["c1on"]))
    wd.append(dmaw(m0_sb[:], io["m0"]))
    # bulk weights wait for the latency-critical input loads to finish so
    # they don't steal HBM bandwidth from the critical path
    for w in wd:
        add_dep_helper(w.ins, d_xpo.ins, sync=True,
                       reason="bulk weights after data loads")

    mm = nc.tensor.matmul
    act = nc.scalar.activation
    tt_ = nc.vector.tensor_tensor
    tcp = nc.vector.tensor_copy

    # ---------------- x^T (own) : 8 PE transposes ----------------
    for tt in range(4):
        for dh in range(2):
            p = pt.tile([128, 128], f32, tag="ptr")
            nc.tensor.transpose(p[:], xc_sb[:, 256 * tt + 128 * dh:
                                            256 * tt + 128 * dh + 128],
                                eyef_sb[:])
            tcp(xT_sb[:, 512 * dh + 128 * tt: 512 * dh + 128 * tt + 128], p[:])

    # ---------------- prefix x_even^T : 12 PE transposes ----------------
    for j in range(6):
        for kt in range(2):
            p = pt.tile([128, 128], bf16, tag="ptr")
            nc.tensor.transpose(p[:], xpe_sb[:, 256 * j + 128 * kt:
                                             256 * j + 128 * kt + 128],
                                eyeb_sb[:])
            tcp(xpeT_sb[:, 768 * kt + 128 * j: 768 * kt + 128 * j + 128],
                p[:])

    # ---------------- own phases -> Q ----------------
    ph_ps = pa.tile([P, T], f32, tag="pa")
    for kt in range(2):
        mm(ph_ps[:], _r(kw_sb[:, P * kt: P * kt + P]),
           _r(xT_sb[:, 512 * kt: 512 * kt + 512]),
           start=(kt == 0), stop=(kt == 1))
    act(t_sb[:], ph_ps[:], AF.Tanh, bias=kbc_sb[:])
    ab_i = act(ta_sb[:], t_sb[:], AF.Abs, bias=zerop_sb[:])
    nc.vector.tensor_scalar(out=ta_sb[:], in0=ta_sb[:], scalar1=-1.0,
                            scalar2=0.5, op0=OP.mult, op1=OP.add)
    qs_i1 = act(qb_sb[0:P, :], ta_sb[:], AF.Sin, bias=zerop_sb[:], scale=PI)
    qs_i2 = act(qb_sb[P:2 * P, :], t_sb[:], AF.Sin, bias=zerop_sb[:], scale=PI)

    # ---------------- prefix phases -> Kpre ----------------
    pp1 = pa.tile([P, 512], f32, tag="pa")
    pp2 = pa.tile([P, 256], f32, tag="pa")
    for kt in range(2):
        mm(pp1[:], kwb_sb[:, P * kt: P * kt + P],
           xpeT_sb[:, 768 * kt: 768 * kt + 512],
           start=(kt == 0), stop=(kt == 1))
    for kt in range(2):
        mm(pp2[:], kwb_sb[:, P * kt: P * kt + P],
           xpeT_sb[:, 768 * kt + 512: 768 * kt + 768],
           start=(kt == 0), stop=(kt == 1))
    tp_i1 = act(tpre_sb[:, 0:512], pp1[:], AF.Tanh, bias=kbc_sb[:])
    tp_i2 = act(tpre_sb[:, 512:768], pp2[:], AF.Tanh, bias=kbc_sb[:])
    # keep both Tanh ops adjacent on ACT before any Sin (table grouping)
    for si in (ab_i, qs_i1, qs_i2):
        for ti in (tp_i1, tp_i2):
            add_dep_helper(si.ins, ti.ins, sync=False,
                           reason="group tanh before sin for act tables")
    act(tpa_sb[:], tpre_sb[:], AF.Abs, bias=zerop_sb[:])
    nc.vector.tensor_scalar(out=tpa_sb[:], in0=tpa_sb[:], scalar1=-1.0,
                            scalar2=0.5, op0=OP.mult, op1=OP.add)
    act(kpre_sb[0:P, :], tpa_sb[:], AF.Sin, bias=zerop_sb[:], scale=PI)
    act(kpre_sb[P:2 * P, :], tpre_sb[:], AF.Sin, bias=zerop_sb[:], scale=PI)

    # ---------------- Kpre^T, G, S ----------------
    for j in range(6):
        p = pt.tile([128, 64], bf16, tag="ptr")
        nc.tensor.transpose(p[:], kpre_sb[:, 128 * j: 128 * j + 128],
                            eyeb_sb[0:64, 0:64])
        tcp(kpreT_sb[:, 64 * j: 64 * j + 64], p[:])
    g_ps = pa.tile([2 * P, D], f32, tag="pa")
    for j in range(6):
        mm(g_ps[:], kpreT_sb[:, 64 * j: 64 * j + 64],
           xpo_sb[:, 256 * j: 256 * j + 256],
           start=(j == 0), stop=(j == 5))
    tcp(g_sb[:], g_ps[:])
    for dh in range(2):
        p = pt.tile([128, 64], f32, tag="ptr")
        nc.tensor.transpose(p[:], g_sb[:, 128 * dh: 128 * dh + 128],
                            eyef_sb[0:64, 0:64])
        tcp(gT_sb[:, 64 * dh: 64 * dh + 64], p[:])
    s_ps = pa.tile([2 * P, D], f32, tag="pa")
    for kt in range(2):
        mm(s_ps[:], _r(gT_sb[:, 64 * kt: 64 * kt + 64]),
           _r(vw_sb[:, 256 * kt: 256 * kt + 256]),
           start=(kt == 0), stop=(kt == 1))
    tcp(s_sb[:], s_ps[:])

    # ---------------- V at odd own tokens ----------------
    for blk in range(2):
        vp = pa.tile([128, D], f32, tag="pa")
        for kt in range(2):
            mm(vp[:], _r(xT_sb[:, 512 * kt + 256 * blk + 1:
                               512 * kt + 256 * blk + 256: 2]),
               _r(vw_sb[:, 256 * kt: 256 * kt + 256]),
               start=(kt == 0), stop=(kt == 1))
        tcp(vodd_sb[:, 256 * blk: 256 * blk + 256], vp[:])

    # ---------------- scores (odd tk only) + causal mask ----------------
    sc0 = pb.tile([128, 512], f32, tag="pb")
    mm(sc0[:], _r(qb_sb[:, 0:255:2]), _r(qb_sb[:]), start=True, stop=True)
    tt_(ss0_sb[:, 0:256], sc0[:, 0:256], m0_sb[:], OP.mult)
    tcp(ss0_sb[:, 256:512], sc0[:, 256:512])
    sc1 = pa.tile([128, 256], f32, tag="pa")
    mm(sc1[:], _r(qb_sb[:, 256:511:2]), _r(qb_sb[:, 256:512]),
       start=True, stop=True)
    tt_(ss1_sb[:], sc1[:], m0_sb[:], OP.mult)

    # ---------------- retrieved^T = V^T s + S^T Q ----------------
    retr = []
    for dh in range(2):
        rp = pb.tile([128, 512], f32, tag="pb")
        mm(rp[:], _r(s_sb[:, 128 * dh: 128 * dh + 128]), _r(qb_sb[:]),
           start=True, stop=False)
        mm(rp[:], _r(vodd_sb[:, 128 * dh: 128 * dh + 128]), _r(ss0_sb[:]),
           start=False, stop=False, skip_group_check=True)
        mm(rp[:, 256:512], _r(vodd_sb[:, 256 + 128 * dh: 256 + 128 * dh + 128]),
           _r(ss1_sb[:]), start=False, stop=True, skip_group_check=True)
        retr.append(rp)

    # ---------------- LN1 (folded) ----------------
    def layer_norm(src_ps, src_sb, sqt, stp, msqp, var_sb, m2v_sb, stdc, rstdc,
                   rstd_sb, q_sb, badd):
        # src_ps: 2 psum tiles [128,512] (or None -> read src_sb);
        # copies to src_sb (+ optional per-partition bias), squares, stats,
        # rstd row + broadcast; returns rstdb psum tile [128,512].
        for dh in range(2):
            sl = slice(512 * dh, 512 * dh + 512)
            if src_ps is not None:
                if badd is None:
                    tcp(src_sb[:, sl], src_ps[dh][:])
                    act(sqt[:, sl], src_ps[dh][:], AF.Square, bias=zero128_sb[:])
                else:
                    nc.vector.tensor_scalar(
                        out=src_sb[:, sl], in0=src_ps[dh][:],
                        scalar1=badd[:, dh: dh + 1], scalar2=None, op0=OP.add)
                    act(sqt[:, sl], src_sb[:, sl], AF.Square, bias=zero128_sb[:])
            else:
                act(sqt[:, sl], src_sb[:, sl], AF.Square, bias=zero128_sb[:])
        for kt in range(2):
            mm(stp[0:1, :], _r(invd_sb[:, 0:1]),
               _r(src_sb[:, 512 * kt: 512 * kt + 512]),
               start=(kt == 0), stop=(kt == 1))
        for kt in range(2):
            mm(msqp[0:1, :], _r(invd_sb[:, 0:1]),
               _r(sqt[:, 512 * kt: 512 * kt + 512]),
               start=(kt == 0), stop=(kt == 1))
        act(m2v_sb[:], stp[0:1, :], AF.Square, bias=zero1_sb[:])
        tt_(var_sb[:], msqp[0:1, :], m2v_sb[:], OP.subtract)
        vc = pt.tile([128, 4], f32, tag="ptr")
        for j in range(4):
            nc.tensor.transpose(vc[:, j: j + 1],
                                var_sb[0:1, 128 * j: 128 * j + 128],
                                eyef_sb[0:1, 0:1])
        # rstd = rsqrt(var + eps): magic-seed Newton, all DVE, no ACT table
        i32 = mybir.dt.int32
        nc.vector.tensor_scalar(out=stdc[:], in0=vc[:], scalar1=1e-5,
                                scalar2=None, op0=OP.add)
        nc.vector.tensor_scalar(out=rstdc[:].bitcast(i32),
                                in0=stdc[:].bitcast(i32), scalar1=1,
                                scalar2=None, op0=OP.logical_shift_right)
        nc.vector.tensor_scalar(out=rstdc[:].bitcast(i32),
                                in0=rstdc[:].bitcast(i32), scalar1=-1,
                                scalar2=0x5F3759DF, op0=OP.mult, op1=OP.add)
        nwt = pa.tile([128, 4], f32, tag="pa2", bufs=1)
        for _ in range(3):
            tt_(nwt[:], rstdc[:], rstdc[:], OP.mult)
            tt_(nwt[:], nwt[:], stdc[:], OP.mult)
            nc.vector.tensor_scalar(out=nwt[:], in0=nwt[:], scalar1=-0.5,
                                    scalar2=1.5, op0=OP.mult, op1=OP.add)
            tt_(rstdc[:], rstdc[:], nwt[:], OP.mult)
        rr = pa.tile([1, T], f32, tag="pa")
        for j in range(4):
            nc.tensor.transpose(rr[0:1, 128 * j: 128 * j + 128],
                                rstdc[:, j: j + 1], eyef_sb[:])
        tcp(rstd_sb[:], rr[:])
        tt_(q_sb[:], stp[0:1, :], rstd_sb[:], OP.mult)
        rb = pb.tile([128, 512], f32, tag="pb")
        mm(rb[:], _r(ones_sb[:]), _r(rstd_sb[:]), start=True, stop=True)
        return rb

    st1 = pa.tile([1, T], f32, tag="pa")
    ms1 = pa.tile([1, T], f32, tag="pa2", bufs=1)
    rb1 = layer_norm(retr, r_sb, sq_sb, st1, ms1, var1_sb, m2_sb, stdc_sb,
                     rstdc_sb, rstd1_sb, q1_sb, None)
    for dh in range(2):
        sl = slice(512 * dh, 512 * dh + 512)
        tt_(rhat_sb[:, sl], r_sb[:, sl], rb1[:], OP.mult)

    # ---------------- W1 + rank-1 mean fix + GELU ----------------
    for m in range(4):
        hp = pb.tile([128, 512], f32, tag="pb")
        for kt in range(2):
            mm(hp[:], w1g_sb[:, 512 * kt + 128 * m: 512 * kt + 128 * m + 128],
               rhat_sb[:, 512 * kt: 512 * kt + 512],
               start=(kt == 0), stop=False)
        mm(hp[:], c1n_sb[0:1, 128 * m: 128 * m + 128], q1_sb[:],
           start=False, stop=True, skip_group_check=True)
        act(h_sb[:, 512 * m: 512 * m + 512], hp[:], AF.Gelu,
            bias=cb1_sb[:, m: m + 1])

    # ---------------- W2 -> refined ----------------
    ref = []
    for dh in range(2):
        fp = pb.tile([128, 512], f32, tag="pb")
        for kt in range(4):
            mm(fp[:], w2_sb[:, 256 * kt + 128 * dh: 256 * kt + 128 * dh + 128],
               h_sb[:, 512 * kt: 512 * kt + 512],
               start=(kt == 0), stop=(kt == 3))
        ref.append(fp)

    # ---------------- LN2 (folded) ----------------
    st2 = pa.tile([1, T], f32, tag="pa")
    ms2 = pa.tile([1, T], f32, tag="pa2", bufs=1)
    rb2 = layer_norm(ref, f_sb, sq2_sb, st2, ms2, var2_sb, m2b_sb, stdc2_sb,
                     rstdc2_sb, rstd2_sb, q2_sb, b2c_sb)
    for dh in range(2):
        sl = slice(512 * dh, 512 * dh + 512)
        tt_(y2_sb[:, sl], f_sb[:, sl], rb2[:], OP.mult)

    # ---------------- Wo (token-major) + rank-1 + residual ----------------
    for tm in range(4):
        op = pa.tile([128, D], f32, tag="pa")
        for dh in range(2):
            mm(op[:], y2_sb[:, 512 * dh + 128 * tm: 512 * dh + 128 * tm + 128],
               wog_sb[:, 256 * dh: 256 * dh + 256],
               start=(dh == 0), stop=False)
        mm(op[:], q2_sb[0:1, 128 * tm: 128 * tm + 128], c1on_sb[:],
           start=False, stop=True, skip_group_check=True)
        tt_(out_sb[:, 256 * tm: 256 * tm + 256], op[:],
            xc_sb[:, 256 * tm: 256 * tm + 256], OP.add)
        dma(outc[128 * tm: 128 * tm + 128, :],
            out_sb[:, 256 * tm: 256 * tm + 256])



    pb.release()
    pa.release()
    pt.release()
    sb.release()


_CACHE = {}


def _get_nc():
    if "nc" not in _CACHE:
        _CACHE["nc"] = _build()
    return _CACHE["nc"]


def _bf(a):
    return np.asarray(a, np.float32).astype(ml_dtypes.bfloat16)


def kernel(**inputs):
    x = np.asarray(inputs["x"], np.float32)
    key_W = np.asarray(inputs["key_W"], np.float32)
    key_b = np.asarray(inputs["key_b"], np.float32)
    val_W = np.asarray(inputs["val_W"], np.float32)
    val_b = np.asarray(inputs["val_b"], np.float32)
    ln1_g = np.asarray(inputs["ln1_g"], np.float32)
    ln1_b = np.asarray(inputs["ln1_b"], np.float32)
    W1 = np.asarray(inputs["W1"], np.float32)
    b1 = np.asarray(inputs["b1"], np.float32)
    W2 = np.asarray(inputs["W2"], np.float32)
    b2 = np.asarray(inputs["b2"], np.float32)
    ln2_g = np.asarray(inputs["ln2_g"], np.float32)
    ln2_b = np.asarray(inputs["ln2_b"], np.float32)
    Wo = np.asarray(inputs["Wo"], np.float32)
    bo = np.asarray(inputs["bo"], np.float32)

    # these are identically zero for this module; the kernel folds them out
    assert np.allclose(val_b, 0.0), "nonzero val_b unsupported"
    assert np.allclose(bo + ln2_b @ Wo, 0.0), "nonzero output bias unsupported"

    w1g = ln1_g[:, None] * W1
    wog = ln2_g[:, None] * Wo
    shared = {
        "kw": key_W, "kwb": _bf(key_W), "kbc": key_b.reshape(P, 1),
        "vw": val_W,
        "w1g": _bf(w1g), "c1n": _bf(-w1g.sum(0, keepdims=True)),
        "cb1": (b1 + ln1_b @ W1).reshape(H, 1),
        "w2": _bf(W2), "b2c": b2.reshape(D, 1),
        "wog": _bf(wog), "c1on": _bf(-wog.sum(0, keepdims=True)),
        "m0": (np.arange(1, 256, 2)[:, None] <=
               np.arange(256)[None, :]).astype(np.float32),
        "eyef": np.eye(128, dtype=np.float32),
        "onesr": np.ones((1, 128), np.float32),
        "invd": np.full((D, 1), 1.0 / D, np.float32)[:128],
        "eyeb": _bf(np.eye(128)),
    }
    in_maps = []
    for i in range(N_CORES):
        b, c = divmod(i, 4)
        l0 = c * T
        npairs = l0 // 2
        xpe = np.zeros((PRE, D), np.float32)
        xpo = np.zeros((PRE, D), np.float32)
        if npairs:
            xpe[:npairs] = x[b, 0:l0 - 1:2]
            xpo[:npairs] = x[b, 1:l0:2]
        in_maps.append({
            "xc": np.ascontiguousarray(x[b, l0:l0 + T]),
            "xpe": _bf(xpe), "xpo": _bf(xpo), **shared,
        })

    nc = _get_nc()
    res = run_bass_kernel_spmd(nc, in_maps, core_ids=list(range(N_CORES)),
                               **_CACHE.get("run_kwargs", {}))
    _CACHE["last_result"] = res
    out = np.empty((B, L, D), np.float32)
    for i in range(N_CORES):
        b, c = divmod(i, 4)
        out[b, c * T:(c + 1) * T] = res.results[i]["outc"]
    return out


# revision 24
# speedup vs baseline: 1.2317x; 1.0592x over previous
"""Trainium2 Bass kernel for nn_HardcodedKVMemoryBlock (8 NeuronCores).

Sharding: core i handles batch b=i//4, sequence chunk c=i%4 (512 tokens).
The (B,L,P,D) cumsum is restructured as causal linear attention:
    retrieved = tril(Q K^T) @ V + Q @ S_prefix
with Q=[cos,sin] phasors (L x 64), V = values at odd positions, and the
cross-chunk carry S_prefix = (K_even^T @ x_odd) @ val_W computed
redundantly per core from a zero-padded prefix (no collectives).
The 1/sqrt(valid*P) normalization cancels inside LayerNorm1 (scale
invariance); ln gains are folded into W1/Wo and means are applied as
rank-1 PE updates, so LN costs no extra full-width element-wise passes
beyond one multiply by the broadcast rstd.
"""

import math
import numpy as np
import ml_dtypes

import concourse.bass as bass
import concourse.tile as tile
from concourse import bacc, mybir
from concourse.bass_utils import run_bass_kernel_spmd

PI = math.pi
B, L, D, P = 2, 2048, 256, 32
T = 512          # own tokens per core
H = 512          # MLP hidden
PRE = 768        # padded prefix pair count (max prefix 1536 tokens / 2)
N_CORES = 8

f32 = mybir.dt.float32
f32r = mybir.dt.float32r
bf16 = mybir.dt.bfloat16
AF = mybir.ActivationFunctionType
OP = mybir.AluOpType


def _r(ap):
    return ap.bitcast(f32r)


def _build():
    nc = bacc.Bacc("TRN2", target_bir_lowering=False, debug=False,
                   num_devices=N_CORES)

    def din(name, shape, dt):
        return nc.dram_tensor(name, shape, dt, kind="ExternalInput").ap()

    xc = din("xc", [T, D], f32)
    xpe = din("xpe", [PRE, D], bf16)
    xpo = din("xpo", [PRE, D], bf16)
    kw = din("kw", [D, P], f32r)
    kwb = din("kwb", [D, P], bf16)
    kbc = din("kbc", [P, 1], f32)
    vw = din("vw", [D, D], f32r)
    w1g = din("w1g", [D, H], bf16)
    c1n = din("c1n", [1, H], bf16)
    cb1 = din("cb1", [H, 1], f32)
    w2 = din("w2", [H, D], bf16)
    b2c = din("b2c", [D, 1], f32)
    wog = din("wog", [D, D], bf16)
    c1on = din("c1on", [1, D], bf16)
    m0 = din("m0", [128, 256], f32)
    onesr = din("onesr", [1, 128], f32r)
    invd = din("invd", [128, 1], f32r)
    eyef = din("eyef", [128, 128], f32)
    eyeb = din("eyeb", [128, 128], bf16)
    outc = nc.dram_tensor("outc", [T, D], f32, kind="ExternalOutput").ap()

    with tile.TileContext(nc) as tc:
        _emit(tc, locals())
    nc.compile()
    return nc


def _emit(tc, io):
    nc = tc.nc
    xc, xpe, xpo = io["xc"], io["xpe"], io["xpo"]
    outc = io["outc"]

    sb = tc.alloc_tile_pool(name="sb", bufs=1)
    pt = tc.alloc_tile_pool(name="pt", bufs=2, space="PSUM")    # transposes
    pa = tc.alloc_tile_pool(name="pa", bufs=2, space="PSUM")    # narrow tiles
    pb = tc.alloc_tile_pool(name="pb", bufs=3, space="PSUM")    # [128,512]

    # ---------------- constant / weight tiles ----------------
    kw_sb = sb.tile([128, 2 * P], f32r)       # ktile kt at cols [P*kt]
    kwb_sb = sb.tile([128, 2 * P], bf16)
    kbc_sb = sb.tile([P, 1], f32)
    vw_sb = sb.tile([128, 512], f32r)         # ktile kt at cols [256*kt]
    w1g_sb = sb.tile([128, 1024], bf16)      # ktile kt at cols [512*kt]
    c1n_sb = sb.tile([1, H], bf16)
    cb1_sb = sb.tile([128, 4], f32)          # mtile m at col m
    w2_sb = sb.tile([128, 1024], bf16)       # ktile kt at cols [256*kt]
    b2c_sb = sb.tile([128, 2], f32)
    wog_sb = sb.tile([128, 512], bf16)       # ktile dh at cols [256*dh]
    c1on_sb = sb.tile([1, D], bf16)
    m0_sb = sb.tile([128, 256], f32)
    eyef_sb = sb.tile([128, 128], f32)
    eyeb_sb = sb.tile([128, 128], bf16)
    ones_sb = sb.tile([1, 128], f32r)
    invd_sb = sb.tile([128, 1], f32r)
    epsb_sb = sb.tile([128, 1], f32)
    halfpi_sb = sb.tile([P, 1], f32)
    zerop_sb = sb.tile([P, 1], f32)
    zero128_sb = sb.tile([128, 1], f32)
    zero1_sb = sb.tile([1, 1], f32)

    dma = nc.sync.dma_start
    dmaw = nc.gpsimd.dma_start
    nc.vector.memset(epsb_sb[:], 1e-5)
    nc.vector.memset(halfpi_sb[:], PI / 2)
    nc.vector.memset(zerop_sb[:], 0.0)
    nc.vector.memset(zero128_sb[:], 0.0)
    nc.vector.memset(zero1_sb[:], 0.0)

    # ---------------- data tiles ----------------
    xc_sb = sb.tile([128, 1024], f32)        # token tile tt at cols [256*tt]
    xT_sb = sb.tile([128, 1024], f32r)        # d-half dh at cols [512*dh]
    xpe_sb = sb.tile([128, 1536], bf16)      # block j at cols [256*j]
    xpo_sb = sb.tile([128, 1536], bf16)
    xpeT_sb = sb.tile([128, 1536], bf16)     # ktile kt at cols [768*kt]
    t_sb = sb.tile([P, T], f32)
    ta_sb = sb.tile([P, T], f32)
    qb_sb = sb.tile([2 * P, T], f32r)         # rows 0:32 cos, 32:64 sin
    tpre_sb = sb.tile([P, PRE], f32)
    tpa_sb = sb.tile([P, PRE], f32)
    kpre_sb = sb.tile([2 * P, PRE], bf16)
    kpreT_sb = sb.tile([128, 6 * 64], bf16)  # block j at cols [64*j]
    g_sb = sb.tile([2 * P, D], f32)
    gT_sb = sb.tile([128, 128], f32r)         # ktile dh at cols [64*dh]
    s_sb = sb.tile([2 * P, D], f32r)
    vodd_sb = sb.tile([128, 512], f32r)       # block blk at cols [256*blk]
    ss0_sb = sb.tile([128, 512], f32r)
    ss1_sb = sb.tile([128, 256], f32r)
    r_sb = sb.tile([128, 1024], f32r)         # retrieved, dh at cols [512*dh]
    sq_sb = sb.tile([128, 1024], f32r)
    rhat_sb = sb.tile([128, 1024], bf16)
    h_sb = sb.tile([128, 2048], bf16)        # mtile m at cols [512*m]
    f_sb = sb.tile([128, 1024], f32r)         # refined
    sq2_sb = sb.tile([128, 1024], f32r)
    y2_sb = sb.tile([128, 1024], bf16)
    out_sb = sb.tile([128, 1024], f32)       # token tile tt at cols [256*tt]

    var1_sb = sb.tile([1, T], f32)
    rstd1_sb = sb.tile([1, T], f32r)
    q1_sb = sb.tile([1, T], bf16)
    m2_sb = sb.tile([1, T], f32)
    stdc_sb = sb.tile([128, 4], f32)
    rstdc_sb = sb.tile([128, 4], f32)
    var2_sb = sb.tile([1, T], f32)
    rstd2_sb = sb.tile([1, T], f32r)
    q2_sb = sb.tile([1, T], bf16)
    m2b_sb = sb.tile([1, T], f32)
    stdc2_sb = sb.tile([128, 4], f32)
    rstdc2_sb = sb.tile([128, 4], f32)


    from concourse.tile import add_dep_helper
    d_xc = dma(xc_sb[:], xc.rearrange("(t p) d -> p t d", p=128))
    dma(eyef_sb[:], io["eyef"])
    d_xpe = dma(xpe_sb[:], xpe.rearrange("(j p) d -> p j d", p=128))
    d_xpo = dma(xpo_sb[:], xpo.rearrange("(j p) d -> p j d", p=128))
    dma(kw_sb[:], io["kw"].rearrange("(k p) q -> p k q", p=128))
    dma(kwb_sb[:], io["kwb"].rearrange("(k p) q -> p k q", p=128))
    dma(kbc_sb[:], io["kbc"])
    dma(eyeb_sb[:], io["eyeb"])
    dma(ones_sb[:], io["onesr"])
    dma(invd_sb[:], io["invd"])
    wd = []
    wd.append(dmaw(vw_sb[:], io["vw"].rearrange("(k p) q -> p k q", p=128)))
    wd.append(dmaw(w1g_sb[:], io["w1g"].rearrange("(k p) q -> p k q", p=128)))
    wd.append(dmaw(c1n_sb[:], io["c1n"]))
    wd.append(dmaw(cb1_sb[:], io["cb1"].rearrange("(m p) o -> p m o", p=128)))
    wd.append(dmaw(w2_sb[:], io["w2"].rearrange("(k p) q -> p k q", p=128)))
    wd.append(dmaw(b2c_sb[:], io["b2c"].rearrange("(m p) o -> p m o", p=128)))
    wd.append(dmaw(wog_sb[:], io["wog"].rearrange("(k p) q -> p k q", p=128)))
    wd.append(dmaw(c1on_sb[:], io["c1on"]))
    wd.append(dmaw(m0_sb[:], io["m0"]))
    # bulk weights wait for the latency-critical input loads to finish so
    # they don't steal HBM bandwidth from the critical path
    for w in wd:
        add_dep_helper(w.ins, d_xpo.ins, sync=True,
                       reason="bulk weights after data loads")

    mm = nc.tensor.matmul
    act = nc.scalar.activation
    tt_ = nc.vector.tensor_tensor
    tcp = nc.vector.tensor_copy

    # ---------------- x^T (own) : 8 PE transposes ----------------
    for tt in range(4):
        for dh in range(2):
            p = pt.tile([128, 128], f32, tag="ptr")
            nc.tensor.transpose(p[:], xc_sb[:, 256 * tt + 128 * dh:
                                            256 * tt + 128 * dh + 128],
                                eyef_sb[:])
            nc.scalar.copy(xT_sb[:, 512 * dh + 128 * tt:
                                 512 * dh + 128 * tt + 128], p[:])

    # ---------------- prefix x_even^T : 12 PE transposes ----------------
    for j in range(6):
        for kt in range(2):
            p = pt.tile([128, 128], bf16, tag="ptr")
            nc.tensor.transpose(p[:], xpe_sb[:, 256 * j + 128 * kt:
                                             256 * j + 128 * kt + 128],
                                eyeb_sb[:])
            tcp(xpeT_sb[:, 768 * kt + 128 * j: 768 * kt + 128 * j + 128],
                p[:])

    # ---------------- own phases -> Q ----------------
    ph_ps = pa.tile([P, T], f32, tag="pa")
    for kt in range(2):
        mm(ph_ps[:], _r(kw_sb[:, P * kt: P * kt + P]),
           _r(xT_sb[:, 512 * kt: 512 * kt + 512]),
           start=(kt == 0), stop=(kt == 1))
    act(t_sb[:], ph_ps[:], AF.Tanh, bias=kbc_sb[:])
    ab_i = act(ta_sb[:], t_sb[:], AF.Abs, bias=zerop_sb[:])
    nc.vector.tensor_scalar(out=ta_sb[:], in0=ta_sb[:], scalar1=-1.0,
                            scalar2=0.5, op0=OP.mult, op1=OP.add)
    qs_i1 = act(qb_sb[0:P, :], ta_sb[:], AF.Sin, bias=zerop_sb[:], scale=PI)
    qs_i2 = act(qb_sb[P:2 * P, :], t_sb[:], AF.Sin, bias=zerop_sb[:], scale=PI)

    # ---------------- prefix phases -> Kpre ----------------
    pp1 = pa.tile([P, 512], f32, tag="pa")
    pp2 = pa.tile([P, 256], f32, tag="pa")
    for kt in range(2):
        mm(pp1[:], kwb_sb[:, P * kt: P * kt + P],
           xpeT_sb[:, 768 * kt: 768 * kt + 512],
           start=(kt == 0), stop=(kt == 1))
    for kt in range(2):
        mm(pp2[:], kwb_sb[:, P * kt: P * kt + P],
           xpeT_sb[:, 768 * kt + 512: 768 * kt + 768],
           start=(kt == 0), stop=(kt == 1))
    tp_i1 = act(tpre_sb[:, 0:512], pp1[:], AF.Tanh, bias=kbc_sb[:])
    tp_i2 = act(tpre_sb[:, 512:768], pp2[:], AF.Tanh, bias=kbc_sb[:])
    # keep both Tanh ops adjacent on ACT before any Sin (table grouping)
    for si in (ab_i, qs_i1, qs_i2):
        for ti in (tp_i1, tp_i2):
            add_dep_helper(si.ins, ti.ins, sync=False,
                           reason="group tanh before sin for act tables")
    act(tpa_sb[:], tpre_sb[:], AF.Abs, bias=zerop_sb[:])
    nc.vector.tensor_scalar(out=tpa_sb[:], in0=tpa_sb[:], scalar1=-1.0,
                            scalar2=0.5, op0=OP.mult, op1=OP.add)
    act(kpre_sb[0:P, :], tpa_sb[:], AF.Sin, bias=zerop_sb[:], scale=PI)
    act(kpre_sb[P:2 * P, :], tpre_sb[:], AF.Sin, bias=zerop_sb[:], scale=PI)

    # ---------------- V at odd own tokens ----------------
    for blk in range(2):
        vp = pa.tile([128, D], f32, tag="pa")
        for kt in range(2):
            mm(vp[:], _r(xT_sb[:, 512 * kt + 256 * blk + 1:
                               512 * kt + 256 * blk + 256: 2]),
               _r(vw_sb[:, 256 * kt: 256 * kt + 256]),
               start=(kt == 0), stop=(kt == 1))
        nc.scalar.copy(vodd_sb[:, 256 * blk: 256 * blk + 256], vp[:])

    # ---------------- scores (odd tk only) + causal mask ----------------
    sc0 = pb.tile([128, 512], f32, tag="pb")
    mm(sc0[:], _r(qb_sb[:, 0:255:2]), _r(qb_sb[:]), start=True, stop=True)
    tt_(ss0_sb[:, 0:256], sc0[:, 0:256], m0_sb[:], OP.mult)
    nc.scalar.copy(ss0_sb[:, 256:512], sc0[:, 256:512])
    sc1 = pa.tile([128, 256], f32, tag="pa")
    mm(sc1[:], _r(qb_sb[:, 256:511:2]), _r(qb_sb[:, 256:512]),
       start=True, stop=True)
    tt_(ss1_sb[:], sc1[:], m0_sb[:], OP.mult)

    # ---------------- Kpre^T, G, S ----------------
    for j in range(6):
        p = pt.tile([128, 64], bf16, tag="ptr")
        nc.tensor.transpose(p[:], kpre_sb[:, 128 * j: 128 * j + 128],
                            eyeb_sb[0:64, 0:64])
        tcp(kpreT_sb[:, 64 * j: 64 * j + 64], p[:])
    g_ps = pa.tile([2 * P, D], f32, tag="pa")
    for j in range(6):
        mm(g_ps[:], kpreT_sb[:, 64 * j: 64 * j + 64],
           xpo_sb[:, 256 * j: 256 * j + 256],
           start=(j == 0), stop=(j == 5))
    tcp(g_sb[:], g_ps[:])
    for dh in range(2):
        p = pt.tile([128, 64], f32, tag="ptr")
        nc.tensor.transpose(p[:], g_sb[:, 128 * dh: 128 * dh + 128],
                            eyef_sb[0:64, 0:64])
        tcp(gT_sb[:, 64 * dh: 64 * dh + 64], p[:])
    s_ps = pa.tile([2 * P, D], f32, tag="pa")
    for kt in range(2):
        mm(s_ps[:], _r(gT_sb[:, 64 * kt: 64 * kt + 64]),
           _r(vw_sb[:, 256 * kt: 256 * kt + 256]),
           start=(kt == 0), stop=(kt == 1))
    tcp(s_sb[:], s_ps[:])

    # ---------------- retrieved^T = V^T s + S^T Q ----------------
    retr = []
    for dh in range(2):
        rp = pb.tile([128, 512], f32, tag="pb")
        mm(rp[:], _r(vodd_sb[:, 128 * dh: 128 * dh + 128]), _r(ss0_sb[:]),
           start=True, stop=False)
        mm(rp[:, 256:512], _r(vodd_sb[:, 256 + 128 * dh: 256 + 128 * dh + 128]),
           _r(ss1_sb[:]), start=False, stop=False, skip_group_check=True)
        mm(rp[:], _r(s_sb[:, 128 * dh: 128 * dh + 128]), _r(qb_sb[:]),
           start=False, stop=True, skip_group_check=True)
        retr.append(rp)

    # ---------------- LN1 (folded) ----------------
    def layer_norm(src_ps, src_sb, sqt, stp, msqp, var_sb, m2v_sb, stdc, rstdc,
                   rstd_sb, q_sb, badd):
        # src_ps: 2 psum tiles [128,512] (or None -> read src_sb);
        # copies to src_sb (+ optional per-partition bias), squares, stats,
        # rstd row + broadcast; returns rstdb psum tile [128,512].
        for dh in range(2):
            sl = slice(512 * dh, 512 * dh + 512)
            if src_ps is not None:
                if badd is None:
                    tcp(src_sb[:, sl], src_ps[dh][:])
                    act(sqt[:, sl], src_ps[dh][:], AF.Square, bias=zero128_sb[:])
                else:
                    nc.vector.tensor_scalar(
                        out=src_sb[:, sl], in0=src_ps[dh][:],
                        scalar1=badd[:, dh: dh + 1], scalar2=None, op0=OP.add)
                    act(sqt[:, sl], src_sb[:, sl], AF.Square, bias=zero128_sb[:])
            else:
                act(sqt[:, sl], src_sb[:, sl], AF.Square, bias=zero128_sb[:])
        for kt in range(2):
            mm(stp[0:1, :], _r(invd_sb[:, 0:1]),
               _r(src_sb[:, 512 * kt: 512 * kt + 512]),
               start=(kt == 0), stop=(kt == 1))
        for kt in range(2):
            mm(msqp[0:1, :], _r(invd_sb[:, 0:1]),
               _r(sqt[:, 512 * kt: 512 * kt + 512]),
               start=(kt == 0), stop=(kt == 1))
        act(m2v_sb[:], stp[0:1, :], AF.Square, bias=zero1_sb[:])
        tt_(var_sb[:], msqp[0:1, :], m2v_sb[:], OP.subtract)
        vc = pt.tile([128, 4], f32, tag="ptr")
        for j in range(4):
            nc.tensor.transpose(vc[:, j: j + 1],
                                var_sb[0:1, 128 * j: 128 * j + 128],
                                eyef_sb[0:1, 0:1])
        # rstd = rsqrt(var + eps): magic-seed Newton, all DVE, no ACT table
        i32 = mybir.dt.int32
        nc.vector.tensor_scalar(out=stdc[:], in0=vc[:], scalar1=1e-5,
                                scalar2=None, op0=OP.add)
        nc.vector.tensor_scalar(out=rstdc[:].bitcast(i32),
                                in0=stdc[:].bitcast(i32), scalar1=1,
                                scalar2=None, op0=OP.logical_shift_right)
        nc.vector.tensor_scalar(out=rstdc[:].bitcast(i32),
                                in0=rstdc[:].bitcast(i32), scalar1=-1,
                                scalar2=0x5F3759DF, op0=OP.mult, op1=OP.add)
        nwt = pa.tile([128, 4], f32, tag="pa2", bufs=1)
        for _ in range(3):
            tt_(nwt[:], rstdc[:], rstdc[:], OP.mult)
            tt_(nwt[:], nwt[:], stdc[:], OP.mult)
            nc.vector.tensor_scalar(out=nwt[:], in0=nwt[:], scalar1=-0.5,
                                    scalar2=1.5, op0=OP.mult, op1=OP.add)
            tt_(rstdc[:], rstdc[:], nwt[:], OP.mult)
        rr = pa.tile([1, T], f32, tag="pa")
        for j in range(4):
            nc.tensor.transpose(rr[0:1, 128 * j: 128 * j + 128],
                                rstdc[:, j: j + 1], eyef_sb[:])
        tcp(rstd_sb[:], rr[:])
        tt_(q_sb[:], stp[0:1, :], rstd_sb[:], OP.mult)
        rb = pb.tile([128, 512], f32, tag="pb")
        mm(rb[:], _r(ones_sb[:]), _r(rstd_sb[:]), start=True, stop=True)
        return rb

    st1 = pa.tile([1, T], f32, tag="pa")
    ms1 = pa.tile([1, T], f32, tag="pa2", bufs=1)
    rb1 = layer_norm(retr, r_sb, sq_sb, st1, ms1, var1_sb, m2_sb, stdc_sb,
                     rstdc_sb, rstd1_sb, q1_sb, None)
    for dh in range(2):
        sl = slice(512 * dh, 512 * dh + 512)
        tt_(rhat_sb[:, sl], r_sb[:, sl], rb1[:], OP.mult)

    # ---------------- W1 + rank-1 mean fix + GELU ----------------
    for m in range(4):
        hp = pb.tile([128, 512], f32, tag="pb")
        for kt in range(2):
            mm(hp[:], w1g_sb[:, 512 * kt + 128 * m: 512 * kt + 128 * m + 128],
               rhat_sb[:, 512 * kt: 512 * kt + 512],
               start=(kt == 0), stop=False)
        mm(hp[:], c1n_sb[0:1, 128 * m: 128 * m + 128], q1_sb[:],
           start=False, stop=True, skip_group_check=True)
        act(h_sb[:, 512 * m: 512 * m + 512], hp[:], AF.Gelu,
            bias=cb1_sb[:, m: m + 1])

    # ---------------- W2 -> refined ----------------
    ref = []
    for dh in range(2):
        fp = pb.tile([128, 512], f32, tag="pb")
        for kt in range(4):
            mm(fp[:], w2_sb[:, 256 * kt + 128 * dh: 256 * kt + 128 * dh + 128],
               h_sb[:, 512 * kt: 512 * kt + 512],
               start=(kt == 0), stop=(kt == 3))
        ref.append(fp)

    # ---------------- LN2 (folded) ----------------
    st2 = pa.tile([1, T], f32, tag="pa")
    ms2 = pa.tile([1, T], f32, tag="pa2", bufs=1)
    rb2 = layer_norm(ref, f_sb, sq2_sb, st2, ms2, var2_sb, m2b_sb, stdc2_sb,
                     rstdc2_sb, rstd2_sb, q2_sb, b2c_sb)
    for dh in range(2):
        sl = slice(512 * dh, 512 * dh + 512)
        tt_(y2_sb[:, sl], f_sb[:, sl], rb2[:], OP.mult)

    # ---------------- Wo (token-major) + rank-1 + residual ----------------
    for tm in range(4):
        op = pa.tile([128, D], f32, tag="pa")
        for dh in range(2):
            mm(op[:], y2_sb[:, 512 * dh + 128 * tm: 512 * dh + 128 * tm + 128],
               wog_sb[:, 256 * dh: 256 * dh + 256],
               start=(dh == 0), stop=False)
        mm(op[:], q2_sb[0:1, 128 * tm: 128 * tm + 128], c1on_sb[:],
           start=False, stop=True, skip_group_check=True)
        tt_(out_sb[:, 256 * tm: 256 * tm + 256], op[:],
            xc_sb[:, 256 * tm: 256 * tm + 256], OP.add)
        dma(outc[128 * tm: 128 * tm + 128, :],
            out_sb[:, 256 * tm: 256 * tm + 256])



    pb.release()
    pa.release()
    pt.release()
    sb.release()


_CACHE = {}


def _get_nc():
    if "nc" not in _CACHE:
        _CACHE["nc"] = _build()
    return _CACHE["nc"]


def _bf(a):
    return np.asarray(a, np.float32).astype(ml_dtypes.bfloat16)


def kernel(**inputs):
    x = np.asarray(inputs["x"], np.float32)
    key_W = np.asarray(inputs["key_W"], np.float32)
    key_b = np.asarray(inputs["key_b"], np.float32)
    val_W = np.asarray(inputs["val_W"], np.float32)
    val_b = np.asarray(inputs["val_b"], np.float32)
    ln1_g = np.asarray(inputs["ln1_g"], np.float32)
    ln1_b = np.asarray(inputs["ln1_b"], np.float32)
    W1 = np.asarray(inputs["W1"], np.float32)
    b1 = np.asarray(inputs["b1"], np.float32)
    W2 = np.asarray(inputs["W2"], np.float32)
    b2 = np.asarray(inputs["b2"], np.float32)
    ln2_g = np.asarray(inputs["ln2_g"], np.float32)
    ln2_b = np.asarray(inputs["ln2_b"], np.float32)
    Wo = np.asarray(inputs["Wo"], np.float32)
    bo = np.asarray(inputs["bo"], np.float32)

    # these are identically zero for this module; the kernel folds them out
    assert np.allclose(val_b, 0.0), "nonzero val_b unsupported"
    assert np.allclose(bo + ln2_b @ Wo, 0.0), "nonzero output bias unsupported"

    w1g = ln1_g[:, None] * W1
    wog = ln2_g[:, None] * Wo
    shared = {
        "kw": key_W, "kwb": _bf(key_W), "kbc": key_b.reshape(P, 1),
        "vw": val_W,
        "w1g": _bf(w1g), "c1n": _bf(-w1g.sum(0, keepdims=True)),
        "cb1": (b1 + ln1_b @ W1).reshape(H, 1),
        "w2": _bf(W2), "b2c": b2.reshape(D, 1),
        "wog": _bf(wog), "c1on": _bf(-wog.sum(0, keepdims=True)),
        "m0": (np.arange(1, 256, 2)[:, None] <=
               np.arange(256)[None, :]).astype(np.float32),
        "eyef": np.eye(128, dtype=np.float32),
        "onesr": np.ones((1, 128), np.float32),
        "invd": np.full((D, 1), 1.0 / D, np.float32)[:128],
        "eyeb": _bf(np.eye(128)),
    }
    in_maps = []
    for i in range(N_CORES):
        b, c = divmod(i, 4)
        l0 = c * T
        npairs = l0 // 2
        xpe = np.zeros((PRE, D), np.float32)
        xpo = np.zeros((PRE, D), np.float32)
        if npairs:
            xpe[:npairs] = x[b, 0:l0 - 1:2]
            xpo[:npairs] = x[b, 1:l0:2]
        in_maps.append({
            "xc": np.ascontiguousarray(x[b, l0:l0 + T]),
            "xpe": _bf(xpe), "xpo": _bf(xpo), **shared,
        })

    nc = _get_nc()
    res = run_bass_kernel_spmd(nc, in_maps, core_ids=list(range(N_CORES)),
                               **_CACHE.get("run_kwargs", {}))
    _CACHE["last_result"] = res
    out = np.empty((B, L, D), np.float32)
    for i in range(N_CORES):
        b, c = divmod(i, 4)
        out[b, c * T:(c + 1) * T] = res.results[i]["outc"]
    return out


# revision 25
# speedup vs baseline: 1.2666x; 1.0284x over previous
"""Trainium2 Bass kernel for nn_HardcodedKVMemoryBlock (8 NeuronCores).

Sharding: core i handles batch b=i//4, sequence chunk c=i%4 (512 tokens).
The (B,L,P,D) cumsum is restructured as causal linear attention:
    retrieved = tril(Q K^T) @ V + Q @ S_prefix
with Q=[cos,sin] phasors (L x 64), V = values at odd positions, and the
cross-chunk carry S_prefix = (K_even^T @ x_odd) @ val_W computed
redundantly per core from a zero-padded prefix (no collectives).
The 1/sqrt(valid*P) normalization cancels inside LayerNorm1 (scale
invariance); ln gains are folded into W1/Wo and means are applied as
rank-1 PE updates, so LN costs no extra full-width element-wise passes
beyond one multiply by the broadcast rstd.
"""

import math
import numpy as np
import ml_dtypes

import concourse.bass as bass
import concourse.tile as tile
from concourse import bacc, mybir
from concourse.bass_utils import run_bass_kernel_spmd

PI = math.pi
B, L, D, P = 2, 2048, 256, 32
T = 512          # own tokens per core
H = 512          # MLP hidden
PRE = 768        # padded prefix pair count (max prefix 1536 tokens / 2)
N_CORES = 8

f32 = mybir.dt.float32
f32r = mybir.dt.float32r
bf16 = mybir.dt.bfloat16
AF = mybir.ActivationFunctionType
OP = mybir.AluOpType


def _r(ap):
    return ap.bitcast(f32r)


def _build():
    nc = bacc.Bacc("TRN2", target_bir_lowering=False, debug=False,
                   num_devices=N_CORES)

    def din(name, shape, dt):
        return nc.dram_tensor(name, shape, dt, kind="ExternalInput").ap()

    xc = din("xc", [T, D], f32)
    xpe = din("xpe", [PRE, D], bf16)
    xpo = din("xpo", [PRE, D], bf16)
    kw = din("kw", [D, P], f32r)
    kwb = din("kwb", [D, P], bf16)
    kbc = din("kbc", [P, 1], f32)
    vw = din("vw", [D, D], f32r)
    w1g = din("w1g", [D, H], bf16)
    c1n = din("c1n", [1, H], bf16)
    cb1 = din("cb1", [H, 1], f32)
    w2 = din("w2", [H, D], bf16)
    b2c = din("b2c", [D, 1], f32)
    wog = din("wog", [D, D], bf16)
    c1on = din("c1on", [1, D], bf16)
    m0 = din("m0", [128, 256], f32)
    onesr = din("onesr", [1, 128], f32r)
    invd = din("invd", [128, 1], f32r)
    eyef = din("eyef", [128, 128], f32)
    eyeb = din("eyeb", [128, 128], bf16)
    outc = nc.dram_tensor("outc", [T, D], f32, kind="ExternalOutput").ap()

    with tile.TileContext(nc) as tc:
        _emit(tc, locals())
    nc.compile()
    return nc


def _emit(tc, io):
    nc = tc.nc
    xc, xpe, xpo = io["xc"], io["xpe"], io["xpo"]
    outc = io["outc"]

    sb = tc.alloc_tile_pool(name="sb", bufs=1)
    pt = tc.alloc_tile_pool(name="pt", bufs=2, space="PSUM")    # transposes
    pa = tc.alloc_tile_pool(name="pa", bufs=2, space="PSUM")    # narrow tiles
    pb = tc.alloc_tile_pool(name="pb", bufs=3, space="PSUM")    # [128,512]

    # ---------------- constant / weight tiles ----------------
    kw_sb = sb.tile([128, 2 * P], f32r)       # ktile kt at cols [P*kt]
    kwb_sb = sb.tile([128, 2 * P], bf16)
    kbc_sb = sb.tile([P, 1], f32)
    vw_sb = sb.tile([128, 512], f32r)         # ktile kt at cols [256*kt]
    w1g_sb = sb.tile([128, 1024], bf16)      # ktile kt at cols [512*kt]
    c1n_sb = sb.tile([1, H], bf16)
    cb1_sb = sb.tile([128, 4], f32)          # mtile m at col m
    w2_sb = sb.tile([128, 1024], bf16)       # ktile kt at cols [256*kt]
    b2c_sb = sb.tile([128, 2], f32)
    wog_sb = sb.tile([128, 512], bf16)       # ktile dh at cols [256*dh]
    c1on_sb = sb.tile([1, D], bf16)
    m0_sb = sb.tile([128, 256], f32)
    eyef_sb = sb.tile([128, 128], f32)
    eyeb_sb = sb.tile([128, 128], bf16)
    ones_sb = sb.tile([1, 128], f32r)
    invd_sb = sb.tile([128, 1], f32r)
    epsb_sb = sb.tile([128, 1], f32)
    halfpi_sb = sb.tile([P, 1], f32)
    zerop_sb = sb.tile([P, 1], f32)
    zero64_sb = sb.tile([2 * P, 1], f32)
    zero128_sb = sb.tile([128, 1], f32)
    zero1_sb = sb.tile([1, 1], f32)

    dma = nc.sync.dma_start
    dmaw = nc.gpsimd.dma_start
    nc.vector.memset(epsb_sb[:], 1e-5)
    nc.vector.memset(halfpi_sb[:], PI / 2)
    nc.vector.memset(zerop_sb[:], 0.0)
    nc.vector.memset(zero64_sb[:], 0.0)
    nc.vector.memset(zero128_sb[:], 0.0)
    nc.vector.memset(zero1_sb[:], 0.0)

    # ---------------- data tiles ----------------
    xc_sb = sb.tile([128, 1024], f32)        # token tile tt at cols [256*tt]
    xT_sb = sb.tile([128, 1024], f32r)        # d-half dh at cols [512*dh]
    xpe_sb = sb.tile([128, 1536], bf16)      # block j at cols [256*j]
    xpo_sb = sb.tile([128, 1536], bf16)
    xpeT_sb = sb.tile([128, 1536], bf16)     # ktile kt at cols [768*kt]
    tv_sb = sb.tile([2 * P, T], f32)
    qb_sb = sb.tile([2 * P, T], f32r)         # rows 0:32 cos, 32:64 sin
    tvp_sb = sb.tile([2 * P, PRE], f32)
    kpre_sb = sb.tile([2 * P, PRE], bf16)
    kpreT_sb = sb.tile([128, 6 * 64], bf16)  # block j at cols [64*j]
    g_sb = sb.tile([2 * P, D], f32)
    gT_sb = sb.tile([128, 128], f32r)         # ktile dh at cols [64*dh]
    s_sb = sb.tile([2 * P, D], f32r)
    vodd_sb = sb.tile([128, 512], f32r)       # block blk at cols [256*blk]
    ss0_sb = sb.tile([128, 512], f32r)
    ss1_sb = sb.tile([128, 256], f32r)
    r_sb = sb.tile([128, 1024], f32r)         # retrieved, dh at cols [512*dh]
    sq_sb = sb.tile([128, 1024], f32r)
    rhat_sb = sb.tile([128, 1024], bf16)
    h_sb = sb.tile([128, 2048], bf16)        # mtile m at cols [512*m]
    f_sb = sb.tile([128, 1024], f32r)         # refined
    sq2_sb = sb.tile([128, 1024], f32r)
    y2_sb = sb.tile([128, 1024], bf16)
    out_sb = sb.tile([128, 1024], f32)       # token tile tt at cols [256*tt]

    var1_sb = sb.tile([1, T], f32)
    rstd1_sb = sb.tile([1, T], f32r)
    q1_sb = sb.tile([1, T], bf16)
    m2_sb = sb.tile([1, T], f32)
    stdc_sb = sb.tile([128, 4], f32)
    rstdc_sb = sb.tile([128, 4], f32)
    var2_sb = sb.tile([1, T], f32)
    rstd2_sb = sb.tile([1, T], f32r)
    q2_sb = sb.tile([1, T], bf16)
    m2b_sb = sb.tile([1, T], f32)
    stdc2_sb = sb.tile([128, 4], f32)
    rstdc2_sb = sb.tile([128, 4], f32)


    from concourse.tile import add_dep_helper
    d_xc = dma(xc_sb[:], xc.rearrange("(t p) d -> p t d", p=128))
    dma(eyef_sb[:], io["eyef"])
    d_xpe = dma(xpe_sb[:], xpe.rearrange("(j p) d -> p j d", p=128))
    d_xpo = dma(xpo_sb[:], xpo.rearrange("(j p) d -> p j d", p=128))
    dma(kw_sb[:], io["kw"].rearrange("(k p) q -> p k q", p=128))
    dma(kwb_sb[:], io["kwb"].rearrange("(k p) q -> p k q", p=128))
    dma(kbc_sb[:], io["kbc"])
    dma(eyeb_sb[:], io["eyeb"])
    dma(ones_sb[:], io["onesr"])
    dma(invd_sb[:], io["invd"])
    wd = []
    wd.append(dmaw(vw_sb[:], io["vw"].rearrange("(k p) q -> p k q", p=128)))
    wd.append(dmaw(w1g_sb[:], io["w1g"].rearrange("(k p) q -> p k q", p=128)))
    wd.append(dmaw(c1n_sb[:], io["c1n"]))
    wd.append(dmaw(cb1_sb[:], io["cb1"].rearrange("(m p) o -> p m o", p=128)))
    wd.append(dmaw(w2_sb[:], io["w2"].rearrange("(k p) q -> p k q", p=128)))
    wd.append(dmaw(b2c_sb[:], io["b2c"].rearrange("(m p) o -> p m o", p=128)))
    wd.append(dmaw(wog_sb[:], io["wog"].rearrange("(k p) q -> p k q", p=128)))
    wd.append(dmaw(c1on_sb[:], io["c1on"]))
    wd.append(dmaw(m0_sb[:], io["m0"]))
    # bulk weights wait for the latency-critical input loads to finish so
    # they don't steal HBM bandwidth from the critical path
    for w in wd:
        add_dep_helper(w.ins, d_xpo.ins, sync=True,
                       reason="bulk weights after data loads")

    mm = nc.tensor.matmul
    act = nc.scalar.activation
    tt_ = nc.vector.tensor_tensor
    tcp = nc.vector.tensor_copy

    # ---------------- x^T (own) : 8 PE transposes ----------------
    for tt in range(4):
        for dh in range(2):
            p = pt.tile([128, 128], f32, tag="ptr")
            nc.tensor.transpose(p[:], xc_sb[:, 256 * tt + 128 * dh:
                                            256 * tt + 128 * dh + 128],
                                eyef_sb[:])
            nc.scalar.copy(xT_sb[:, 512 * dh + 128 * tt:
                                 512 * dh + 128 * tt + 128], p[:])

    # ---------------- prefix x_even^T : 12 PE transposes ----------------
    for j in range(6):
        for kt in range(2):
            p = pt.tile([128, 128], bf16, tag="ptr")
            nc.tensor.transpose(p[:], xpe_sb[:, 256 * j + 128 * kt:
                                             256 * j + 128 * kt + 128],
                                eyeb_sb[:])
            tcp(xpeT_sb[:, 768 * kt + 128 * j: 768 * kt + 128 * j + 128],
                p[:])

    # ---------------- own phases -> Q ----------------
    ph_ps = pa.tile([P, T], f32, tag="pa")
    for kt in range(2):
        mm(ph_ps[:], _r(kw_sb[:, P * kt: P * kt + P]),
           _r(xT_sb[:, 512 * kt: 512 * kt + 512]),
           start=(kt == 0), stop=(kt == 1))
    act(tv_sb[P:2 * P, :], ph_ps[:], AF.Tanh, bias=kbc_sb[:])
    ab_i = act(tv_sb[0:P, :], tv_sb[P:2 * P, :], AF.Abs, bias=zerop_sb[:])
    nc.vector.tensor_scalar(out=tv_sb[0:P, :], in0=tv_sb[0:P, :], scalar1=-1.0,
                            scalar2=0.5, op0=OP.mult, op1=OP.add)
    qs_i1 = act(qb_sb[:], tv_sb[:], AF.Sin, bias=zero64_sb[:], scale=PI)
    qs_i2 = qs_i1

    # ---------------- prefix phases -> Kpre ----------------
    pp1 = pa.tile([P, 512], f32, tag="pa")
    pp2 = pa.tile([P, 256], f32, tag="pa")
    for kt in range(2):
        mm(pp1[:], kwb_sb[:, P * kt: P * kt + P],
           xpeT_sb[:, 768 * kt: 768 * kt + 512],
           start=(kt == 0), stop=(kt == 1))
    for kt in range(2):
        mm(pp2[:], kwb_sb[:, P * kt: P * kt + P],
           xpeT_sb[:, 768 * kt + 512: 768 * kt + 768],
           start=(kt == 0), stop=(kt == 1))
    tp_i1 = act(tvp_sb[P:2 * P, 0:512], pp1[:], AF.Tanh, bias=kbc_sb[:])
    tp_i2 = act(tvp_sb[P:2 * P, 512:768], pp2[:], AF.Tanh, bias=kbc_sb[:])
    # keep both Tanh ops adjacent on ACT before any Sin (table grouping)
    for si in (ab_i, qs_i1, qs_i2):
        for ti in (tp_i1, tp_i2):
            add_dep_helper(si.ins, ti.ins, sync=False,
                           reason="group tanh before sin for act tables")
    act(tvp_sb[0:P, :], tvp_sb[P:2 * P, :], AF.Abs, bias=zerop_sb[:])
    nc.vector.tensor_scalar(out=tvp_sb[0:P, :], in0=tvp_sb[0:P, :],
                            scalar1=-1.0, scalar2=0.5, op0=OP.mult, op1=OP.add)
    act(kpre_sb[:], tvp_sb[:], AF.Sin, bias=zero64_sb[:], scale=PI)

    # ---------------- V at odd own tokens ----------------
    for blk in range(2):
        vp = pa.tile([128, D], f32, tag="pa")
        for kt in range(2):
            mm(vp[:], _r(xT_sb[:, 512 * kt + 256 * blk + 1:
                               512 * kt + 256 * blk + 256: 2]),
               _r(vw_sb[:, 256 * kt: 256 * kt + 256]),
               start=(kt == 0), stop=(kt == 1))
        nc.scalar.copy(vodd_sb[:, 256 * blk: 256 * blk + 256], vp[:])

    # ---------------- scores (odd tk only) + causal mask ----------------
    sc0 = pb.tile([128, 512], f32, tag="pb")
    mm(sc0[:], _r(qb_sb[:, 0:255:2]), _r(qb_sb[:]), start=True, stop=True)
    tt_(ss0_sb[:, 0:256], sc0[:, 0:256], m0_sb[:], OP.mult)
    nc.scalar.copy(ss0_sb[:, 256:512], sc0[:, 256:512])
    sc1 = pa.tile([128, 256], f32, tag="pa")
    mm(sc1[:], _r(qb_sb[:, 256:511:2]), _r(qb_sb[:, 256:512]),
       start=True, stop=True)
    tt_(ss1_sb[:], sc1[:], m0_sb[:], OP.mult)

    # ---------------- Kpre^T, G, S ----------------
    for j in range(6):
        p = pt.tile([128, 64], bf16, tag="ptr")
        nc.tensor.transpose(p[:], kpre_sb[:, 128 * j: 128 * j + 128],
                            eyeb_sb[0:64, 0:64])
        tcp(kpreT_sb[:, 64 * j: 64 * j + 64], p[:])
    g_ps = pa.tile([2 * P, D], f32, tag="pa")
    for j in range(6):
        mm(g_ps[:], kpreT_sb[:, 64 * j: 64 * j + 64],
           xpo_sb[:, 256 * j: 256 * j + 256],
           start=(j == 0), stop=(j == 5))
    tcp(g_sb[:], g_ps[:])
    for dh in range(2):
        p = pt.tile([128, 64], f32, tag="ptr")
        nc.tensor.transpose(p[:], g_sb[:, 128 * dh: 128 * dh + 128],
                            eyef_sb[0:64, 0:64])
        tcp(gT_sb[:, 64 * dh: 64 * dh + 64], p[:])
    s_ps = pa.tile([2 * P, D], f32, tag="pa")
    for kt in range(2):
        mm(s_ps[:], _r(gT_sb[:, 64 * kt: 64 * kt + 64]),
           _r(vw_sb[:, 256 * kt: 256 * kt + 256]),
           start=(kt == 0), stop=(kt == 1))
    tcp(s_sb[:], s_ps[:])

    # ---------------- retrieved^T = V^T s + S^T Q ----------------
    retr = []
    for dh in range(2):
        rp = pb.tile([128, 512], f32, tag="pb")
        mm(rp[:], _r(vodd_sb[:, 128 * dh: 128 * dh + 128]), _r(ss0_sb[:]),
           start=True, stop=False)
        mm(rp[:, 256:512], _r(vodd_sb[:, 256 + 128 * dh: 256 + 128 * dh + 128]),
           _r(ss1_sb[:]), start=False, stop=False, skip_group_check=True)
        mm(rp[:], _r(s_sb[:, 128 * dh: 128 * dh + 128]), _r(qb_sb[:]),
           start=False, stop=True, skip_group_check=True)
        retr.append(rp)

    # ---------------- LN1 (folded) ----------------
    def layer_norm(src_ps, src_sb, sqt, stp, msqp, var_sb, m2v_sb, stdc, rstdc,
                   rstd_sb, q_sb, badd):
        # src_ps: 2 psum tiles [128,512] (or None -> read src_sb);
        # copies to src_sb (+ optional per-partition bias), squares, stats,
        # rstd row + broadcast; returns rstdb psum tile [128,512].
        for dh in range(2):
            sl = slice(512 * dh, 512 * dh + 512)
            if src_ps is not None:
                if badd is None:
                    tcp(src_sb[:, sl], src_ps[dh][:])
                    act(sqt[:, sl], src_ps[dh][:], AF.Square, bias=zero128_sb[:])
                else:
                    nc.vector.tensor_scalar(
                        out=src_sb[:, sl], in0=src_ps[dh][:],
                        scalar1=badd[:, dh: dh + 1], scalar2=None, op0=OP.add)
                    act(sqt[:, sl], src_sb[:, sl], AF.Square, bias=zero128_sb[:])
            else:
                act(sqt[:, sl], src_sb[:, sl], AF.Square, bias=zero128_sb[:])
        for kt in range(2):
            mm(stp[0:1, :], _r(invd_sb[:, 0:1]),
               _r(src_sb[:, 512 * kt: 512 * kt + 512]),
               start=(kt == 0), stop=(kt == 1))
        for kt in range(2):
            mm(msqp[0:1, :], _r(invd_sb[:, 0:1]),
               _r(sqt[:, 512 * kt: 512 * kt + 512]),
               start=(kt == 0), stop=(kt == 1))
        act(m2v_sb[:], stp[0:1, :], AF.Square, bias=zero1_sb[:])
        nc.vector.scalar_tensor_tensor(
            out=var_sb[:], in0=msqp[0:1, :], scalar=1e-5, in1=m2v_sb[:],
            op0=OP.add, op1=OP.subtract)
        vc = pt.tile([128, 4], f32, tag="ptr")
        for j in range(4):
            nc.tensor.transpose(vc[:, j: j + 1],
                                var_sb[0:1, 128 * j: 128 * j + 128],
                                eyef_sb[0:1, 0:1])
        # rstd = rsqrt(var + eps): magic-seed Newton, all DVE, no ACT table
        i32 = mybir.dt.int32
        nc.vector.tensor_copy(stdc[:], vc[:])
        nc.vector.tensor_scalar(out=rstdc[:].bitcast(i32),
                                in0=stdc[:].bitcast(i32), scalar1=1,
                                scalar2=None, op0=OP.logical_shift_right)
        nc.vector.tensor_scalar(out=rstdc[:].bitcast(i32),
                                in0=rstdc[:].bitcast(i32), scalar1=-1,
                                scalar2=0x5F3759DF, op0=OP.mult, op1=OP.add)
        nwt = pa.tile([128, 4], f32, tag="pa2", bufs=1)
        for _ in range(2):
            tt_(nwt[:], rstdc[:], rstdc[:], OP.mult)
            tt_(nwt[:], nwt[:], stdc[:], OP.mult)
            nc.vector.tensor_scalar(out=nwt[:], in0=nwt[:], scalar1=-0.5,
                                    scalar2=1.5, op0=OP.mult, op1=OP.add)
            tt_(rstdc[:], rstdc[:], nwt[:], OP.mult)
        rr = pa.tile([1, T], f32, tag="pa")
        for j in range(4):
            nc.tensor.transpose(rr[0:1, 128 * j: 128 * j + 128],
                                rstdc[:, j: j + 1], eyef_sb[:])
        tcp(rstd_sb[:], rr[:])
        tt_(q_sb[:], stp[0:1, :], rstd_sb[:], OP.mult)
        rb = pb.tile([128, 512], f32, tag="pb")
        mm(rb[:], _r(ones_sb[:]), _r(rstd_sb[:]), start=True, stop=True)
        return rb

    st1 = pa.tile([1, T], f32, tag="pa")
    ms1 = pa.tile([1, T], f32, tag="pa2", bufs=1)
    rb1 = layer_norm(retr, r_sb, sq_sb, st1, ms1, var1_sb, m2_sb, stdc_sb,
                     rstdc_sb, rstd1_sb, q1_sb, None)
    for dh in range(2):
        sl = slice(512 * dh, 512 * dh + 512)
        tt_(rhat_sb[:, sl], r_sb[:, sl], rb1[:], OP.mult)

    # ---------------- W1 + rank-1 mean fix + GELU ----------------
    for m in range(4):
        hp = pb.tile([128, 512], f32, tag="pb")
        for kt in range(2):
            mm(hp[:], w1g_sb[:, 512 * kt + 128 * m: 512 * kt + 128 * m + 128],
               rhat_sb[:, 512 * kt: 512 * kt + 512],
               start=(kt == 0), stop=False)
        mm(hp[:], c1n_sb[0:1, 128 * m: 128 * m + 128], q1_sb[:],
           start=False, stop=True, skip_group_check=True)
        act(h_sb[:, 512 * m: 512 * m + 512], hp[:], AF.Gelu,
            bias=cb1_sb[:, m: m + 1])

    # ---------------- W2 -> refined ----------------
    ref = []
    for dh in range(2):
        fp = pb.tile([128, 512], f32, tag="pb")
        for kt in range(4):
            mm(fp[:], w2_sb[:, 256 * kt + 128 * dh: 256 * kt + 128 * dh + 128],
               h_sb[:, 512 * kt: 512 * kt + 512],
               start=(kt == 0), stop=(kt == 3))
        ref.append(fp)

    # ---------------- LN2 (folded) ----------------
    st2 = pa.tile([1, T], f32, tag="pa")
    ms2 = pa.tile([1, T], f32, tag="pa2", bufs=1)
    rb2 = layer_norm(ref, f_sb, sq2_sb, st2, ms2, var2_sb, m2b_sb, stdc2_sb,
                     rstdc2_sb, rstd2_sb, q2_sb, b2c_sb)
    for dh in range(2):
        sl = slice(512 * dh, 512 * dh + 512)
        tt_(y2_sb[:, sl], f_sb[:, sl], rb2[:], OP.mult)

    # ---------------- Wo (token-major) + rank-1 + residual ----------------
    for tm in range(4):
        op = pa.tile([128, D], f32, tag="pa")
        for dh in range(2):
            mm(op[:], y2_sb[:, 512 * dh + 128 * tm: 512 * dh + 128 * tm + 128],
               wog_sb[:, 256 * dh: 256 * dh + 256],
               start=(dh == 0), stop=False)
        mm(op[:], q2_sb[0:1, 128 * tm: 128 * tm + 128], c1on_sb[:],
           start=False, stop=True, skip_group_check=True)
        tt_(out_sb[:, 256 * tm: 256 * tm + 256], op[:],
            xc_sb[:, 256 * tm: 256 * tm + 256], OP.add)
        dma(outc[128 * tm: 128 * tm + 128, :],
            out_sb[:, 256 * tm: 256 * tm + 256])



    pb.release()
    pa.release()
    pt.release()
    sb.release()


_CACHE = {}


def _get_nc():
    if "nc" not in _CACHE:
        _CACHE["nc"] = _build()
    return _CACHE["nc"]


def _bf(a):
    return np.asarray(a, np.float32).astype(ml_dtypes.bfloat16)


def kernel(**inputs):
    x = np.asarray(inputs["x"], np.float32)
    key_W = np.asarray(inputs["key_W"], np.float32)
    key_b = np.asarray(inputs["key_b"], np.float32)
    val_W = np.asarray(inputs["val_W"], np.float32)
    val_b = np.asarray(inputs["val_b"], np.float32)
    ln1_g = np.asarray(inputs["ln1_g"], np.float32)
    ln1_b = np.asarray(inputs["ln1_b"], np.float32)
    W1 = np.asarray(inputs["W1"], np.float32)
    b1 = np.asarray(inputs["b1"], np.float32)
    W2 = np.asarray(inputs["W2"], np.float32)
    b2 = np.asarray(inputs["b2"], np.float32)
    ln2_g = np.asarray(inputs["ln2_g"], np.float32)
    ln2_b = np.asarray(inputs["ln2_b"], np.float32)
    Wo = np.asarray(inputs["Wo"], np.float32)
    bo = np.asarray(inputs["bo"], np.float32)

    # these are identically zero for this module; the kernel folds them out
    assert np.allclose(val_b, 0.0), "nonzero val_b unsupported"
    assert np.allclose(bo + ln2_b @ Wo, 0.0), "nonzero output bias unsupported"

    w1g = ln1_g[:, None] * W1
    wog = ln2_g[:, None] * Wo
    shared = {
        "kw": key_W, "kwb": _bf(key_W), "kbc": key_b.reshape(P, 1),
        "vw": val_W,
        "w1g": _bf(w1g), "c1n": _bf(-w1g.sum(0, keepdims=True)),
        "cb1": (b1 + ln1_b @ W1).reshape(H, 1),
        "w2": _bf(W2), "b2c": b2.reshape(D, 1),
        "wog": _bf(wog), "c1on": _bf(-wog.sum(0, keepdims=True)),
        "m0": (np.arange(1, 256, 2)[:, None] <=
               np.arange(256)[None, :]).astype(np.float32),
        "eyef": np.eye(128, dtype=np.float32),
        "onesr": np.ones((1, 128), np.float32),
        "invd": np.full((D, 1), 1.0 / D, np.float32)[:128],
        "eyeb": _bf(np.eye(128)),
    }
    in_maps = []
    for i in range(N_CORES):
        b, c = divmod(i, 4)
        l0 = c * T
        npairs = l0 // 2
        xpe = np.zeros((PRE, D), np.float32)
        xpo = np.zeros((PRE, D), np.float32)
        if npairs:
            xpe[:npairs] = x[b, 0:l0 - 1:2]
            xpo[:npairs] = x[b, 1:l0:2]
        in_maps.append({
            "xc": np.ascontiguousarray(x[b, l0:l0 + T]),
            "xpe": _bf(xpe), "xpo": _bf(xpo), **shared,
        })

    nc = _get_nc()
    res = run_bass_kernel_spmd(nc, in_maps, core_ids=list(range(N_CORES)),
                               **_CACHE.get("run_kwargs", {}))
    _CACHE["last_result"] = res
    out = np.empty((B, L, D), np.float32)
    for i in range(N_CORES):
        b, c = divmod(i, 4)
        out[b, c * T:(c + 1) * T] = res.results[i]["outc"]
    return out


# revision 31
# speedup vs baseline: 1.3187x; 1.0411x over previous
"""Trainium2 Bass kernel for nn_HardcodedKVMemoryBlock (8 NeuronCores).

Sharding: core i handles batch b=i//4, sequence chunk c=i%4 (512 tokens).
The (B,L,P,D) cumsum is restructured as causal linear attention:
    retrieved = tril(Q K^T) @ V + Q @ S_prefix
with Q=[cos,sin] phasors (L x 64), V = values at odd positions, and the
cross-chunk carry S_prefix = (K_even^T @ x_odd) @ val_W computed
redundantly per core from a zero-padded prefix (no collectives).
The 1/sqrt(valid*P) normalization cancels inside LayerNorm1 (scale
invariance); ln gains are folded into W1/Wo and means are applied as
rank-1 PE updates, so LN costs no extra full-width element-wise passes
beyond one multiply by the broadcast rstd.
"""

import math
import numpy as np
import ml_dtypes

import concourse.bass as bass
import concourse.tile as tile
from concourse import bacc, mybir
from concourse.bass_utils import run_bass_kernel_spmd

PI = math.pi
B, L, D, P = 2, 2048, 256, 32
T = 512          # own tokens per core
H = 512          # MLP hidden
PRE = 768        # padded prefix pair count (max prefix 1536 tokens / 2)
N_CORES = 8

f32 = mybir.dt.float32
f32r = mybir.dt.float32r
bf16 = mybir.dt.bfloat16
AF = mybir.ActivationFunctionType
OP = mybir.AluOpType


def _r(ap):
    return ap.bitcast(f32r)


def _build():
    nc = bacc.Bacc("TRN2", target_bir_lowering=False, debug=False,
                   num_devices=N_CORES)

    def din(name, shape, dt):
        return nc.dram_tensor(name, shape, dt, kind="ExternalInput").ap()

    xc = din("xc", [T, D], f32)
    xpe = din("xpe", [PRE, D], bf16)
    xpo = din("xpo", [PRE, D], bf16)
    kw = din("kw", [D, P], f32r)
    kwb = din("kwb", [D, P], bf16)
    kbc = din("kbc", [P, 1], f32)
    vw = din("vw", [D, D], f32r)
    w1g = din("w1g", [D, H], bf16)
    c1n = din("c1n", [1, H], bf16)
    cb1 = din("cb1", [H, 1], f32)
    w2 = din("w2", [H, D], bf16)
    b2c = din("b2c", [D, 1], f32)
    wog = din("wog", [D, D], bf16)
    c1on = din("c1on", [1, D], bf16)
    m0 = din("m0", [128, 256], f32)
    onesr = din("onesr", [1, 128], f32r)
    invd = din("invd", [128, 1], f32r)
    invdb = din("invdb", [128, 1], bf16)
    eyef = din("eyef", [128, 128], f32)
    eyeb = din("eyeb", [128, 128], bf16)
    outc = nc.dram_tensor("outc", [T, D], f32, kind="ExternalOutput").ap()

    with tile.TileContext(nc) as tc:
        _emit(tc, locals())
    nc.compile()
    return nc


def _emit(tc, io):
    nc = tc.nc
    xc, xpe, xpo = io["xc"], io["xpe"], io["xpo"]
    outc = io["outc"]

    sb = tc.alloc_tile_pool(name="sb", bufs=1)
    pt = tc.alloc_tile_pool(name="pt", bufs=2, space="PSUM")    # transposes
    pa = tc.alloc_tile_pool(name="pa", bufs=2, space="PSUM")    # narrow tiles
    pb = tc.alloc_tile_pool(name="pb", bufs=3, space="PSUM")    # [128,512]

    # ---------------- constant / weight tiles ----------------
    kw_sb = sb.tile([128, 2 * P], f32r)       # ktile kt at cols [P*kt]
    kwb_sb = sb.tile([128, 2 * P], bf16)
    kbc_sb = sb.tile([P, 1], f32)
    vw_sb = sb.tile([128, 512], f32r)         # ktile kt at cols [256*kt]
    w1g_sb = sb.tile([128, 1024], bf16)      # ktile kt at cols [512*kt]
    c1n_sb = sb.tile([1, H], bf16)
    cb1_sb = sb.tile([128, 4], f32)          # mtile m at col m
    w2_sb = sb.tile([128, 1024], bf16)       # ktile kt at cols [256*kt]
    b2c_sb = sb.tile([128, 2], f32)
    wog_sb = sb.tile([128, 512], bf16)       # ktile dh at cols [256*dh]
    c1on_sb = sb.tile([1, D], bf16)
    m0_sb = sb.tile([128, 256], f32)
    eyef_sb = sb.tile([128, 128], f32)
    eyeb_sb = sb.tile([128, 128], bf16)
    ones_sb = sb.tile([1, 128], f32r)
    invd_sb = sb.tile([128, 1], f32r)
    invdb_sb = sb.tile([128, 1], bf16)
    epsb_sb = sb.tile([128, 1], f32)
    halfpi_sb = sb.tile([P, 1], f32)
    zerop_sb = sb.tile([P, 1], f32)
    zero64_sb = sb.tile([2 * P, 1], f32)
    zero128_sb = sb.tile([128, 1], f32)
    zero1_sb = sb.tile([1, 1], f32)

    dma = nc.sync.dma_start
    dmaw = nc.gpsimd.dma_start
    nc.vector.memset(epsb_sb[:], 1e-5)
    nc.vector.memset(halfpi_sb[:], PI / 2)
    nc.vector.memset(zerop_sb[:], 0.0)
    nc.vector.memset(zero64_sb[:], 0.0)
    nc.vector.memset(zero128_sb[:], 0.0)
    nc.vector.memset(zero1_sb[:], 0.0)

    # ---------------- data tiles ----------------
    xc_sb = sb.tile([128, 1024], f32)        # token tile tt at cols [256*tt]
    xT_sb = sb.tile([128, 1024], f32r)        # d-half dh at cols [512*dh]
    xpe_sb = sb.tile([128, 1536], bf16)      # block j at cols [256*j]
    xpo_sb = sb.tile([128, 1536], bf16)
    xpeT_sb = sb.tile([128, 1536], bf16)     # ktile kt at cols [768*kt]
    tv_sb = sb.tile([2 * P, T], f32)
    qb_sb = sb.tile([2 * P, T], f32r)         # rows 0:32 cos, 32:64 sin
    tvp_sb = sb.tile([2 * P, PRE], f32)
    kpre_sb = sb.tile([2 * P, PRE], bf16)
    kpreT_sb = sb.tile([128, 6 * 64], bf16)  # block j at cols [64*j]
    g_sb = sb.tile([2 * P, D], f32)
    gT_sb = sb.tile([128, 128], f32r)         # ktile dh at cols [64*dh]
    s_sb = sb.tile([2 * P, D], f32r)
    vodd_sb = sb.tile([128, 512], f32r)       # block blk at cols [256*blk]
    ss0_sb = sb.tile([128, 512], f32r)
    ss1_sb = sb.tile([128, 256], f32r)
    cross_sb = sb.tile([128, 1024], f32)
    r_sb = sb.tile([128, 1024], bf16)         # retrieved, dh at cols [512*dh]
    sq_sb = sb.tile([128, 1024], f32r)
    h_sb = sb.tile([128, 2048], bf16)        # mtile m at cols [512*m]
    f_sb = sb.tile([128, 1024], bf16)         # refined
    sq2_sb = sb.tile([128, 1024], f32r)
    out_sb = sb.tile([128, 1024], f32)       # token tile tt at cols [256*tt]

    var1_sb = sb.tile([1, T], f32)
    rstd1_sb = sb.tile([1, T], f32r)
    mean1_sb = sb.tile([1, T], bf16)
    rb1s_sb = sb.tile([128, T], f32)
    hi_sb = sb.tile([128, 2048], bf16)
    m2_sb = sb.tile([1, T], f32)
    stdc_sb = sb.tile([128, 4], f32)
    rstdc_sb = sb.tile([128, 4], f32)
    var2_sb = sb.tile([1, T], f32)
    mean2_sb = sb.tile([1, T], bf16)
    m2b_sb = sb.tile([1, T], f32)
    stdc2_sb = sb.tile([128, 4], f32)
    rstdc2_sb = sb.tile([128, 4], f32)


    from concourse.tile import add_dep_helper
    d_xc = dma(xc_sb[:], xc.rearrange("(t p) d -> p t d", p=128))
    dma(eyef_sb[:], io["eyef"])
    d_xpe = dma(xpe_sb[:], xpe.rearrange("(j p) d -> p j d", p=128))
    d_xpo = dma(xpo_sb[:], xpo.rearrange("(j p) d -> p j d", p=128))
    dma(kw_sb[:], io["kw"].rearrange("(k p) q -> p k q", p=128))
    dma(kwb_sb[:], io["kwb"].rearrange("(k p) q -> p k q", p=128))
    dma(kbc_sb[:], io["kbc"])
    dma(eyeb_sb[:], io["eyeb"])
    dma(ones_sb[:], io["onesr"])
    dma(invd_sb[:], io["invd"])
    dma(invdb_sb[:], io["invdb"])
    wd = []
    wd.append(dmaw(vw_sb[:], io["vw"].rearrange("(k p) q -> p k q", p=128)))
    wd.append(dmaw(w1g_sb[:], io["w1g"].rearrange("(k p) q -> p k q", p=128)))
    wd.append(dmaw(c1n_sb[:], io["c1n"]))
    wd.append(dmaw(cb1_sb[:], io["cb1"].rearrange("(m p) o -> p m o", p=128)))
    wd.append(dmaw(w2_sb[:], io["w2"].rearrange("(k p) q -> p k q", p=128)))
    wd.append(dmaw(b2c_sb[:], io["b2c"].rearrange("(m p) o -> p m o", p=128)))
    wd.append(dmaw(wog_sb[:], io["wog"].rearrange("(k p) q -> p k q", p=128)))
    wd.append(dmaw(c1on_sb[:], io["c1on"]))
    wd.append(dmaw(m0_sb[:], io["m0"]))
    # bulk weights wait for the latency-critical input loads to finish so
    # they don't steal HBM bandwidth from the critical path
    for w in wd:
        add_dep_helper(w.ins, d_xpo.ins, sync=True,
                       reason="bulk weights after data loads")

    mm = nc.tensor.matmul
    act = nc.scalar.activation
    tt_ = nc.vector.tensor_tensor
    tcp = nc.vector.tensor_copy

    # ---------------- x^T (own) : 8 PE transposes ----------------
    for tt in range(4):
        for dh in range(2):
            p = pt.tile([128, 128], f32, tag="ptr")
            nc.tensor.transpose(p[:], xc_sb[:, 256 * tt + 128 * dh:
                                            256 * tt + 128 * dh + 128],
                                eyef_sb[:])
            nc.scalar.copy(xT_sb[:, 512 * dh + 128 * tt:
                                 512 * dh + 128 * tt + 128], p[:])

    # ---------------- prefix x_even^T : 12 PE transposes ----------------
    for j in range(6):
        for kt in range(2):
            p = pt.tile([128, 128], bf16, tag="ptr")
            nc.tensor.transpose(p[:], xpe_sb[:, 256 * j + 128 * kt:
                                             256 * j + 128 * kt + 128],
                                eyeb_sb[:])
            tcp(xpeT_sb[:, 768 * kt + 128 * j: 768 * kt + 128 * j + 128],
                p[:])

    # ---------------- own phases -> Q ----------------
    ph_ps = pa.tile([P, T], f32, tag="pa")
    for kt in range(2):
        mm(ph_ps[:], _r(kw_sb[:, P * kt: P * kt + P]),
           _r(xT_sb[:, 512 * kt: 512 * kt + 512]),
           start=(kt == 0), stop=(kt == 1))
    act(tv_sb[P:2 * P, :], ph_ps[:], AF.Tanh, bias=kbc_sb[:])
    ab_i = act(tv_sb[0:P, :], tv_sb[P:2 * P, :], AF.Abs, bias=zerop_sb[:])
    nc.vector.tensor_scalar(out=tv_sb[0:P, :], in0=tv_sb[0:P, :], scalar1=-1.0,
                            scalar2=0.5, op0=OP.mult, op1=OP.add)
    qs_i1 = act(qb_sb[:], tv_sb[:], AF.Sin, bias=zero64_sb[:], scale=PI)
    qs_i2 = qs_i1

    # ---------------- prefix phases -> Kpre ----------------
    pp1 = pa.tile([P, 512], f32, tag="pa")
    pp2 = pa.tile([P, 256], f32, tag="pa")
    for kt in range(2):
        mm(pp1[:], kwb_sb[:, P * kt: P * kt + P],
           xpeT_sb[:, 768 * kt: 768 * kt + 512],
           start=(kt == 0), stop=(kt == 1))
    for kt in range(2):
        mm(pp2[:], kwb_sb[:, P * kt: P * kt + P],
           xpeT_sb[:, 768 * kt + 512: 768 * kt + 768],
           start=(kt == 0), stop=(kt == 1))
    tp_i1 = act(tvp_sb[P:2 * P, 0:512], pp1[:], AF.Tanh, bias=kbc_sb[:])
    tp_i2 = act(tvp_sb[P:2 * P, 512:768], pp2[:], AF.Tanh, bias=kbc_sb[:])
    # keep both Tanh ops adjacent on ACT before any Sin (table grouping)
    for si in (ab_i, qs_i1, qs_i2):
        for ti in (tp_i1, tp_i2):
            add_dep_helper(si.ins, ti.ins, sync=False,
                           reason="group tanh before sin for act tables")
    act(tvp_sb[0:P, :], tvp_sb[P:2 * P, :], AF.Abs, bias=zerop_sb[:])
    nc.vector.tensor_scalar(out=tvp_sb[0:P, :], in0=tvp_sb[0:P, :],
                            scalar1=-1.0, scalar2=0.5, op0=OP.mult, op1=OP.add)
    act(kpre_sb[:], tvp_sb[:], AF.Sin, bias=zero64_sb[:], scale=PI)

    # ---------------- V at odd own tokens ----------------
    for blk in range(2):
        vp = pa.tile([128, D], f32, tag="pa")
        for kt in range(2):
            mm(vp[:], _r(xT_sb[:, 512 * kt + 256 * blk + 1:
                               512 * kt + 256 * blk + 256: 2]),
               _r(vw_sb[:, 256 * kt: 256 * kt + 256]),
               start=(kt == 0), stop=(kt == 1))
        nc.scalar.copy(vodd_sb[:, 256 * blk: 256 * blk + 256], vp[:])

    # ---------------- scores (odd tk only) + causal mask ----------------
    sc0 = pb.tile([128, 512], f32, tag="pb")
    mm(sc0[:], _r(qb_sb[:, 0:255:2]), _r(qb_sb[:]), start=True, stop=True)
    tt_(ss0_sb[:, 0:256], sc0[:, 0:256], m0_sb[:], OP.mult)
    nc.scalar.copy(ss0_sb[:, 256:512], sc0[:, 256:512])
    sc1 = pa.tile([128, 256], f32, tag="pa")
    mm(sc1[:], _r(qb_sb[:, 256:511:2]), _r(qb_sb[:, 256:512]),
       start=True, stop=True)
    tt_(ss1_sb[:], sc1[:], m0_sb[:], OP.mult)

    # ---------------- Kpre^T, G, S ----------------
    for j in range(6):
        p = pt.tile([128, 64], bf16, tag="ptr")
        nc.tensor.transpose(p[:], kpre_sb[:, 128 * j: 128 * j + 128],
                            eyeb_sb[0:64, 0:64])
        tcp(kpreT_sb[:, 64 * j: 64 * j + 64], p[:])
    g_ps = pa.tile([2 * P, D], f32, tag="pa")
    for j in range(6):
        mm(g_ps[:], kpreT_sb[:, 64 * j: 64 * j + 64],
           xpo_sb[:, 256 * j: 256 * j + 256],
           start=(j == 0), stop=(j == 5))
    tcp(g_sb[:], g_ps[:])
    for dh in range(2):
        p = pt.tile([128, 64], f32, tag="ptr")
        nc.tensor.transpose(p[:], g_sb[:, 128 * dh: 128 * dh + 128],
                            eyef_sb[0:64, 0:64])
        tcp(gT_sb[:, 64 * dh: 64 * dh + 64], p[:])
    s_ps = pa.tile([2 * P, D], f32, tag="pa")
    for kt in range(2):
        mm(s_ps[:], _r(gT_sb[:, 64 * kt: 64 * kt + 64]),
           _r(vw_sb[:, 256 * kt: 256 * kt + 256]),
           start=(kt == 0), stop=(kt == 1))
    tcp(s_sb[:], s_ps[:])

    # ---------------- retrieved^T = V^T s + S^T Q ----------------
    retr = []
    retr_cross = []
    for dh in range(2):
        rp = pb.tile([128, 512], f32, tag="pb")
        mm(rp[:, 0:256], _r(vodd_sb[:, 128 * dh: 128 * dh + 128]),
           _r(ss0_sb[:, 0:256]), start=True, stop=True)
        mm(rp[:, 256:512], _r(vodd_sb[:, 128 * dh: 128 * dh + 128]),
           _r(ss0_sb[:, 256:512]), start=True, stop=False)
        mm(rp[:, 256:512], _r(vodd_sb[:, 256 + 128 * dh: 256 + 128 * dh + 128]),
           _r(ss1_sb[:]), start=False, stop=True)
        cp = pa.tile([128, 512], f32, tag="pa")
        mm(cp[:], _r(s_sb[:, 128 * dh: 128 * dh + 128]), _r(qb_sb[:]),
           start=True, stop=True)
        nc.scalar.copy(cross_sb[:, 512 * dh: 512 * dh + 512], cp[:])
        retr.append(rp)
        retr_cross.append(cross_sb[:, 512 * dh: 512 * dh + 512])

    # ---------------- LN1 (folded) ----------------
    def layer_norm(src_ps, src_sb, sqt, stp, msqp, var_sb, m2v_sb, stdc, rstdc,
                   mean_sb, badd, src_add=None):
        # copies src psum -> src_sb (bf16, + optional per-partition bias),
        # squares, stats; produces the mean row (bf16 sbuf, for the rank-1
        # mean fix) and rstd as columns [128,4] via magic-seed Newton.
        for dh in range(2):
            sl = slice(512 * dh, 512 * dh + 512)
            if src_add is not None:
                tt_(src_sb[:, sl], src_ps[dh][:], src_add[dh], OP.add)
            elif badd is None:
                tcp(src_sb[:, sl], src_ps[dh][:])
            else:
                nc.vector.tensor_scalar(
                    out=src_sb[:, sl], in0=src_ps[dh][:],
                    scalar1=badd[:, dh: dh + 1], scalar2=None, op0=OP.add)
            act(sqt[:, sl], src_sb[:, sl], AF.Square, bias=zero128_sb[:])
        for kt in range(2):
            mm(stp[0:1, :], invdb_sb[:, 0:1],
               src_sb[:, 512 * kt: 512 * kt + 512],
               start=(kt == 0), stop=(kt == 1))
        for kt in range(2):
            mm(msqp[0:1, :], _r(invd_sb[:, 0:1]),
               _r(sqt[:, 512 * kt: 512 * kt + 512]),
               start=(kt == 0), stop=(kt == 1))
        tcp(mean_sb[:], stp[0:1, :])
        act(m2v_sb[:], stp[0:1, :], AF.Square, bias=zero1_sb[:])
        nc.vector.scalar_tensor_tensor(
            out=var_sb[:], in0=msqp[0:1, :], scalar=1e-5, in1=m2v_sb[:],
            op0=OP.add, op1=OP.subtract)
        vc = pt.tile([128, 4], f32, tag="ptr")
        for j in range(4):
            nc.tensor.transpose(vc[:, j: j + 1],
                                var_sb[0:1, 128 * j: 128 * j + 128],
                                eyef_sb[0:1, 0:1])
        # rstd = rsqrt(var + eps): magic-seed Newton, all DVE, no ACT table
        i32 = mybir.dt.int32
        nc.vector.tensor_copy(stdc[:], vc[:])
        nc.vector.tensor_scalar(out=rstdc[:].bitcast(i32),
                                in0=stdc[:].bitcast(i32), scalar1=1,
                                scalar2=None, op0=OP.logical_shift_right)
        nc.vector.tensor_scalar(out=rstdc[:].bitcast(i32),
                                in0=rstdc[:].bitcast(i32), scalar1=-1,
                                scalar2=0x5F3759DF, op0=OP.mult, op1=OP.add)
        nwt = pa.tile([128, 4], f32, tag="pa2", bufs=1)
        for _ in range(2):
            tt_(nwt[:], rstdc[:], rstdc[:], OP.mult)
            nc.vector.scalar_tensor_tensor(
                out=nwt[:], in0=nwt[:], scalar=-0.5, in1=stdc[:],
                op0=OP.mult, op1=OP.mult)
            nc.vector.tensor_scalar(out=nwt[:], in0=nwt[:], scalar1=1.5,
                                    scalar2=None, op0=OP.add)
            tt_(rstdc[:], rstdc[:], nwt[:], OP.mult)

    st1 = pa.tile([1, T], f32, tag="pa")
    ms1 = pa.tile([1, T], f32, tag="pa2", bufs=1)
    layer_norm(retr, r_sb, sq_sb, st1, ms1, var1_sb, m2_sb, stdc_sb,
               rstdc_sb, mean1_sb, None, src_add=retr_cross)
    # rstd1 broadcast row [128, T] (psum -> sbuf via scalar engine)
    rr = pa.tile([1, T], f32, tag="pa")
    for j in range(4):
        nc.tensor.transpose(rr[0:1, 128 * j: 128 * j + 128],
                            rstdc_sb[:, j: j + 1], eyef_sb[:])
    tcp(rstd1_sb[:], rr[:])
    rb1 = pb.tile([128, 512], f32, tag="pb")
    mm(rb1[:], _r(ones_sb[:]), _r(rstd1_sb[:]), start=True, stop=True)
    nc.scalar.copy(rb1s_sb[:], rb1[:])

    # ---------------- W1 on raw r (rstd folded after) + GELU ----------------
    # (W1g^T r - mean*c1) * rstd == W1g^T(LN1(r)) ; matmuls don't wait on rstd
    for m in range(4):
        hp = pb.tile([128, 512], f32, tag="pb")
        for kt in range(2):
            mm(hp[:], w1g_sb[:, 512 * kt + 128 * m: 512 * kt + 128 * m + 128],
               r_sb[:, 512 * kt: 512 * kt + 512],
               start=(kt == 0), stop=False)
        mm(hp[:], c1n_sb[0:1, 128 * m: 128 * m + 128], mean1_sb[:],
           start=False, stop=True)
        tt_(hi_sb[:, 512 * m: 512 * m + 512], hp[:], rb1s_sb[:], OP.mult)
        act(h_sb[:, 512 * m: 512 * m + 512],
            hi_sb[:, 512 * m: 512 * m + 512], AF.Gelu,
            bias=cb1_sb[:, m: m + 1])

    # ---------------- W2 -> refined ----------------
    ref = []
    for dh in range(2):
        fp = pb.tile([128, 512], f32, tag="pb")
        for kt in range(4):
            mm(fp[:], w2_sb[:, 256 * kt + 128 * dh: 256 * kt + 128 * dh + 128],
               h_sb[:, 512 * kt: 512 * kt + 512],
               start=(kt == 0), stop=(kt == 3))
        ref.append(fp)

    # ---------------- LN2 (folded) ----------------
    st2 = pa.tile([1, T], f32, tag="pa")
    ms2 = pa.tile([1, T], f32, tag="pa2", bufs=1)
    layer_norm(ref, f_sb, sq2_sb, st2, ms2, var2_sb, m2b_sb, stdc2_sb,
               rstdc2_sb, mean2_sb, b2c_sb)

    # ---------------- Wo (token-major) + rank-1 + fused rstd2+residual ------
    # out = (Wog^T f - mean2*c1o) * rstd2[l] + x ; rstd2 is per-partition in
    # token-major space, so the LN2 apply fuses into the residual op.
    for tm in range(4):
        op = pa.tile([128, D], f32, tag="pa")
        for dh in range(2):
            mm(op[:], f_sb[:, 512 * dh + 128 * tm: 512 * dh + 128 * tm + 128],
               wog_sb[:, 256 * dh: 256 * dh + 256],
               start=(dh == 0), stop=False)
        mm(op[:], mean2_sb[0:1, 128 * tm: 128 * tm + 128], c1on_sb[:],
           start=False, stop=True)
        nc.vector.scalar_tensor_tensor(
            out=out_sb[:, 256 * tm: 256 * tm + 256], in0=op[:],
            scalar=rstdc2_sb[:, tm: tm + 1],
            in1=xc_sb[:, 256 * tm: 256 * tm + 256],
            op0=OP.mult, op1=OP.add)
        dma(outc[128 * tm: 128 * tm + 128, :],
            out_sb[:, 256 * tm: 256 * tm + 256])



    pb.release()
    pa.release()
    pt.release()
    sb.release()


_CACHE = {}


def _get_nc():
    if "nc" not in _CACHE:
        _CACHE["nc"] = _build()
    return _CACHE["nc"]


def _bf(a):
    return np.asarray(a, np.float32).astype(ml_dtypes.bfloat16)


def kernel(**inputs):
    x = np.asarray(inputs["x"], np.float32)
    key_W = np.asarray(inputs["key_W"], np.float32)
    key_b = np.asarray(inputs["key_b"], np.float32)
    val_W = np.asarray(inputs["val_W"], np.float32)
    val_b = np.asarray(inputs["val_b"], np.float32)
    ln1_g = np.asarray(inputs["ln1_g"], np.float32)
    ln1_b = np.asarray(inputs["ln1_b"], np.float32)
    W1 = np.asarray(inputs["W1"], np.float32)
    b1 = np.asarray(inputs["b1"], np.float32)
    W2 = np.asarray(inputs["W2"], np.float32)
    b2 = np.asarray(inputs["b2"], np.float32)
    ln2_g = np.asarray(inputs["ln2_g"], np.float32)
    ln2_b = np.asarray(inputs["ln2_b"], np.float32)
    Wo = np.asarray(inputs["Wo"], np.float32)
    bo = np.asarray(inputs["bo"], np.float32)

    # these are identically zero for this module; the kernel folds them out
    assert np.allclose(val_b, 0.0), "nonzero val_b unsupported"
    assert np.allclose(bo + ln2_b @ Wo, 0.0), "nonzero output bias unsupported"

    w1g = ln1_g[:, None] * W1
    wog = ln2_g[:, None] * Wo
    shared = {
        "kw": key_W, "kwb": _bf(key_W), "kbc": key_b.reshape(P, 1),
        "vw": val_W,
        "w1g": _bf(w1g), "c1n": _bf(-w1g.sum(0, keepdims=True)),
        "cb1": (b1 + ln1_b @ W1).reshape(H, 1),
        "w2": _bf(W2), "b2c": b2.reshape(D, 1),
        "wog": _bf(wog), "c1on": _bf(-wog.sum(0, keepdims=True)),
        "m0": (np.arange(1, 256, 2)[:, None] <=
               np.arange(256)[None, :]).astype(np.float32),
        "eyef": np.eye(128, dtype=np.float32),
        "onesr": np.ones((1, 128), np.float32),
        "invd": np.full((128, 1), 1.0 / D, np.float32),
        "invdb": _bf(np.full((128, 1), 1.0 / D, np.float32)),
        "eyeb": _bf(np.eye(128)),
    }
    in_maps = []
    for i in range(N_CORES):
        b, c = divmod(i, 4)
        l0 = c * T
        npairs = l0 // 2
        xpe = np.zeros((PRE, D), np.float32)
        xpo = np.zeros((PRE, D), np.float32)
        if npairs:
            xpe[:npairs] = x[b, 0:l0 - 1:2]
            xpo[:npairs] = x[b, 1:l0:2]
        in_maps.append({
            "xc": np.ascontiguousarray(x[b, l0:l0 + T]),
            "xpe": _bf(xpe), "xpo": _bf(xpo), **shared,
        })

    nc = _get_nc()
    res = run_bass_kernel_spmd(nc, in_maps, core_ids=list(range(N_CORES)),
                               **_CACHE.get("run_kwargs", {}))
    _CACHE["last_result"] = res
    out = np.empty((B, L, D), np.float32)
    for i in range(N_CORES):
        b, c = divmod(i, 4)
        out[b, c * T:(c + 1) * T] = res.results[i]["outc"]
    return out


# revision 34
# speedup vs baseline: 1.4013x; 1.0626x over previous
"""Trainium2 Bass kernel for nn_HardcodedKVMemoryBlock (8 NeuronCores).

Sharding: core i handles batch b=i//4, sequence chunk c=i%4 (512 tokens).
The (B,L,P,D) cumsum is restructured as causal linear attention:
    retrieved = tril(Q K^T) @ V + Q @ S_prefix
with Q=[cos,sin] phasors (L x 64), V = values at odd positions, and the
cross-chunk carry S_prefix = (K_even^T @ x_odd) @ val_W computed
redundantly per core from a zero-padded prefix (no collectives).
The 1/sqrt(valid*P) normalization cancels inside LayerNorm1 (scale
invariance); ln gains are folded into W1/Wo and means are applied as
rank-1 PE updates, so LN costs no extra full-width element-wise passes
beyond one multiply by the broadcast rstd.
"""

import math
import numpy as np
import ml_dtypes

import concourse.bass as bass
import concourse.tile as tile
from concourse import bacc, mybir
from concourse.bass_utils import run_bass_kernel_spmd

PI = math.pi
B, L, D, P = 2, 2048, 256, 32
T = 512          # own tokens per core
H = 512          # MLP hidden
PRE = 768        # padded prefix pair count (max prefix 1536 tokens / 2)
N_CORES = 8

f32 = mybir.dt.float32
f32r = mybir.dt.float32r
bf16 = mybir.dt.bfloat16
AF = mybir.ActivationFunctionType
OP = mybir.AluOpType


def _r(ap):
    return ap.bitcast(f32r)


def _build():
    nc = bacc.Bacc("TRN2", target_bir_lowering=False, debug=False,
                   num_devices=N_CORES)

    def din(name, shape, dt):
        return nc.dram_tensor(name, shape, dt, kind="ExternalInput").ap()

    xc = din("xc", [T, D], f32)
    xpe = din("xpe", [PRE, D], bf16)
    xpo = din("xpo", [PRE, D], bf16)
    kw = din("kw", [D, P], f32r)
    kwb = din("kwb", [D, P], bf16)
    kbc = din("kbc", [P, 1], f32)
    vw = din("vw", [D, D], f32r)
    w1g = din("w1g", [D, H], bf16)
    c1n = din("c1n", [1, H], bf16)
    cb1 = din("cb1", [H, 1], f32)
    w2 = din("w2", [H, D], bf16)
    b2c = din("b2c", [D, 1], f32)
    wog = din("wog", [D, D], bf16)
    c1on = din("c1on", [1, D], bf16)
    m0 = din("m0", [128, 256], f32)
    onesr = din("onesr", [1, 128], f32r)
    invd = din("invd", [128, 1], f32r)
    invdb = din("invdb", [128, 1], bf16)
    eyef = din("eyef", [128, 128], f32)
    eyeb = din("eyeb", [128, 128], bf16)
    outc = nc.dram_tensor("outc", [T, D], f32, kind="ExternalOutput").ap()

    with tile.TileContext(nc) as tc:
        _emit(tc, locals())
    nc.compile()
    return nc


def _emit(tc, io):
    nc = tc.nc
    xc, xpe, xpo = io["xc"], io["xpe"], io["xpo"]
    outc = io["outc"]

    sb = tc.alloc_tile_pool(name="sb", bufs=1)
    pt = tc.alloc_tile_pool(name="pt", bufs=2, space="PSUM")    # transposes
    pa = tc.alloc_tile_pool(name="pa", bufs=2, space="PSUM")    # narrow tiles
    pb = tc.alloc_tile_pool(name="pb", bufs=3, space="PSUM")    # [128,512]

    # ---------------- constant / weight tiles ----------------
    kw_sb = sb.tile([128, 2 * P], f32r)       # ktile kt at cols [P*kt]
    kwb_sb = sb.tile([128, 2 * P], bf16)
    kbc_sb = sb.tile([P, 1], f32)
    vw_sb = sb.tile([128, 512], f32r)         # ktile kt at cols [256*kt]
    w1g_sb = sb.tile([128, 1024], bf16)      # ktile kt at cols [512*kt]
    c1n_sb = sb.tile([1, H], bf16)
    cb1_sb = sb.tile([128, 4], f32)          # mtile m at col m
    w2_sb = sb.tile([128, 1024], bf16)       # ktile kt at cols [256*kt]
    b2c_sb = sb.tile([128, 2], f32)
    wog_sb = sb.tile([128, 512], bf16)       # ktile dh at cols [256*dh]
    c1on_sb = sb.tile([1, D], bf16)
    m0_sb = sb.tile([128, 256], f32)
    eyef_sb = sb.tile([128, 128], f32)
    eyeb_sb = sb.tile([128, 128], bf16)
    ones_sb = sb.tile([1, 128], f32r)
    invd_sb = sb.tile([128, 1], f32r)
    invdb_sb = sb.tile([128, 1], bf16)
    epsb_sb = sb.tile([128, 1], f32)
    halfpi_sb = sb.tile([P, 1], f32)
    zerop_sb = sb.tile([P, 1], f32)
    zero64_sb = sb.tile([2 * P, 1], f32)
    zero128_sb = sb.tile([128, 1], f32)
    zero1_sb = sb.tile([1, 1], f32)

    dma = nc.sync.dma_start
    dmaw = nc.gpsimd.dma_start
    nc.vector.memset(epsb_sb[:], 1e-5)
    nc.vector.memset(halfpi_sb[:], PI / 2)
    nc.vector.memset(zerop_sb[:], 0.0)
    nc.vector.memset(zero64_sb[:], 0.0)
    nc.vector.memset(zero128_sb[:], 0.0)
    nc.vector.memset(zero1_sb[:], 0.0)

    # ---------------- data tiles ----------------
    xc_sb = sb.tile([128, 1024], f32)        # token tile tt at cols [256*tt]
    xT_sb = sb.tile([128, 1024], f32r)        # d-half dh at cols [512*dh]
    xpe_sb = sb.tile([128, 1536], bf16)      # block j at cols [256*j]
    xpo_sb = sb.tile([128, 1536], bf16)
    xpeT_sb = sb.tile([128, 1536], bf16)     # ktile kt at cols [768*kt]
    tv_sb = sb.tile([2 * P, T], f32)
    qb_sb = sb.tile([2 * P, T], f32r)         # rows 0:32 cos, 32:64 sin
    tvp_sb = sb.tile([2 * P, PRE], f32)
    kpre_sb = sb.tile([2 * P, PRE], bf16)
    kpreT_sb = sb.tile([128, 6 * 64], bf16)  # block j at cols [64*j]
    gT_sb = sb.tile([128, 128], f32r)         # ktile dh at cols [64*dh]
    s_sb = sb.tile([2 * P, D], f32r)
    vodd_sb = sb.tile([128, 512], f32r)       # block blk at cols [256*blk]
    ss0_sb = sb.tile([128, 512], f32r)
    ss1_sb = sb.tile([128, 256], f32r)
    cross_sb = sb.tile([128, 1024], f32)
    r_sb = sb.tile([128, 1024], bf16)         # retrieved, dh at cols [512*dh]
    sq_sb = sb.tile([128, 1024], f32r)
    h_sb = sb.tile([128, 2048], bf16)        # mtile m at cols [512*m]
    f_sb = sb.tile([128, 1024], bf16)         # refined
    sq2_sb = sb.tile([128, 1024], f32r)
    out_sb = sb.tile([128, 1024], f32)       # token tile tt at cols [256*tt]

    var1_sb = sb.tile([1, T], f32)
    rstd1_sb = sb.tile([1, T], f32r)
    mean1_sb = sb.tile([1, T], bf16)
    rb1s_sb = sb.tile([128, T], f32)
    hi_sb = sb.tile([128, 2048], bf16)
    m2_sb = sb.tile([1, T], f32)
    stdc_sb = sb.tile([128, 4], f32)
    rstdc_sb = sb.tile([128, 4], f32)
    var2_sb = sb.tile([1, T], f32)
    mean2_sb = sb.tile([1, T], bf16)
    m2b_sb = sb.tile([1, T], f32)
    stdc2_sb = sb.tile([128, 4], f32)
    rstdc2_sb = sb.tile([128, 4], f32)


    from concourse.tile import add_dep_helper
    d_xc = dma(xc_sb[:], xc.rearrange("(t p) d -> p t d", p=128))
    dma(eyef_sb[:], io["eyef"])
    d_xpe = dma(xpe_sb[:], xpe.rearrange("(j p) d -> p j d", p=128))
    d_xpo = dma(xpo_sb[:], xpo.rearrange("(j p) d -> p j d", p=128))
    dma(kw_sb[:], io["kw"].rearrange("(k p) q -> p k q", p=128))
    dma(kwb_sb[:], io["kwb"].rearrange("(k p) q -> p k q", p=128))
    dma(kbc_sb[:], io["kbc"])
    dma(eyeb_sb[:], io["eyeb"])
    dma(ones_sb[:], io["onesr"])
    dma(invd_sb[:], io["invd"])
    dma(invdb_sb[:], io["invdb"])
    wd = []
    wd.append(dmaw(vw_sb[:], io["vw"].rearrange("(k p) q -> p k q", p=128)))
    wd.append(dmaw(w1g_sb[:], io["w1g"].rearrange("(k p) q -> p k q", p=128)))
    wd.append(dmaw(c1n_sb[:], io["c1n"]))
    wd.append(dmaw(cb1_sb[:], io["cb1"].rearrange("(m p) o -> p m o", p=128)))
    wd.append(dmaw(w2_sb[:], io["w2"].rearrange("(k p) q -> p k q", p=128)))
    wd.append(dmaw(b2c_sb[:], io["b2c"].rearrange("(m p) o -> p m o", p=128)))
    wd.append(dmaw(wog_sb[:], io["wog"].rearrange("(k p) q -> p k q", p=128)))
    wd.append(dmaw(c1on_sb[:], io["c1on"]))
    wd.append(dmaw(m0_sb[:], io["m0"]))
    # bulk weights wait for the latency-critical input loads to finish so
    # they don't steal HBM bandwidth from the critical path
    for w in wd:
        add_dep_helper(w.ins, d_xpo.ins, sync=True,
                       reason="bulk weights after data loads")

    mm = nc.tensor.matmul
    act = nc.scalar.activation
    tt_ = nc.vector.tensor_tensor
    tcp = nc.vector.tensor_copy

    # ---------------- x^T (own) : 8 PE transposes ----------------
    for tt in range(4):
        for dh in range(2):
            p = pt.tile([128, 128], f32, tag="ptr")
            nc.tensor.transpose(p[:], xc_sb[:, 256 * tt + 128 * dh:
                                            256 * tt + 128 * dh + 128],
                                eyef_sb[:])
            dst = xT_sb[:, 512 * dh + 128 * tt: 512 * dh + 128 * tt + 128]
            (tcp if tt % 2 == 0 else nc.scalar.copy)(dst, p[:])

    # ---------------- prefix x_even^T : 12 PE transposes ----------------
    for j in range(6):
        for kt in range(2):
            p = pt.tile([128, 128], bf16, tag="ptr")
            nc.tensor.transpose(p[:], xpe_sb[:, 256 * j + 128 * kt:
                                             256 * j + 128 * kt + 128],
                                eyeb_sb[:])
            tcp(xpeT_sb[:, 768 * kt + 128 * j: 768 * kt + 128 * j + 128],
                p[:])

    # ---------------- own phases -> Q ----------------
    ph_ps = pa.tile([P, T], f32, tag="pa")
    for kt in range(2):
        mm(ph_ps[:], _r(kw_sb[:, P * kt: P * kt + P]),
           _r(xT_sb[:, 512 * kt: 512 * kt + 512]),
           start=(kt == 0), stop=(kt == 1))
    act(tv_sb[P:2 * P, :], ph_ps[:], AF.Tanh, bias=kbc_sb[:])

    # ---------------- prefix phases -> Kpre ----------------
    pp1 = pa.tile([P, 512], f32, tag="pa")
    pp2 = pa.tile([P, 256], f32, tag="pa")
    for kt in range(2):
        mm(pp1[:], kwb_sb[:, P * kt: P * kt + P],
           xpeT_sb[:, 768 * kt: 768 * kt + 512],
           start=(kt == 0), stop=(kt == 1))
    for kt in range(2):
        mm(pp2[:], kwb_sb[:, P * kt: P * kt + P],
           xpeT_sb[:, 768 * kt + 512: 768 * kt + 768],
           start=(kt == 0), stop=(kt == 1))
    tp_i1 = act(tvp_sb[P:2 * P, 0:512], pp1[:], AF.Tanh, bias=kbc_sb[:])
    tp_i2 = act(tvp_sb[P:2 * P, 512:768], pp2[:], AF.Tanh, bias=kbc_sb[:])
    act(tvp_sb[0:P, :], tvp_sb[P:2 * P, :], AF.Abs, bias=zerop_sb[:])
    nc.vector.tensor_scalar(out=tvp_sb[0:P, :], in0=tvp_sb[0:P, :],
                            scalar1=-1.0, scalar2=0.5, op0=OP.mult, op1=OP.add)
    act(kpre_sb[:], tvp_sb[:], AF.Sin, bias=zero64_sb[:], scale=PI)
    act(tv_sb[0:P, :], tv_sb[P:2 * P, :], AF.Abs, bias=zerop_sb[:])
    nc.vector.tensor_scalar(out=tv_sb[0:P, :], in0=tv_sb[0:P, :], scalar1=-1.0,
                            scalar2=0.5, op0=OP.mult, op1=OP.add)
    act(qb_sb[:], tv_sb[:], AF.Sin, bias=zero64_sb[:], scale=PI)

    # ---------------- V at odd own tokens ----------------
    for blk in range(2):
        vp = pa.tile([128, D], f32, tag="pa")
        for kt in range(2):
            mm(vp[:], _r(xT_sb[:, 512 * kt + 256 * blk + 1:
                               512 * kt + 256 * blk + 256: 2]),
               _r(vw_sb[:, 256 * kt: 256 * kt + 256]),
               start=(kt == 0), stop=(kt == 1))
        nc.scalar.copy(vodd_sb[:, 256 * blk: 256 * blk + 256], vp[:])

    # ---------------- scores (odd tk only) + causal mask ----------------
    sc0 = pb.tile([128, 512], f32, tag="pb")
    mm(sc0[:], _r(qb_sb[:, 0:255:2]), _r(qb_sb[:]), start=True, stop=True)
    tt_(ss0_sb[:, 0:256], sc0[:, 0:256], m0_sb[:], OP.mult)
    nc.scalar.copy(ss0_sb[:, 256:512], sc0[:, 256:512])
    sc1 = pa.tile([128, 256], f32, tag="pa")
    mm(sc1[:], _r(qb_sb[:, 256:511:2]), _r(qb_sb[:, 256:512]),
       start=True, stop=True)
    tt_(ss1_sb[:], sc1[:], m0_sb[:], OP.mult)

    # ---------------- Kpre^T, G, S ----------------
    for j in range(6):
        p = pt.tile([128, 64], bf16, tag="ptr")
        nc.tensor.transpose(p[:], kpre_sb[:, 128 * j: 128 * j + 128],
                            eyeb_sb[0:64, 0:64])
        tcp(kpreT_sb[:, 64 * j: 64 * j + 64], p[:])
    for dh in range(2):
        gp = pa.tile([128, 64], f32, tag="pa2", bufs=1)
        for j in range(6):
            mm(gp[:], xpo_sb[:, 256 * j + 128 * dh: 256 * j + 128 * dh + 128],
               kpreT_sb[:, 64 * j: 64 * j + 64],
               start=(j == 0), stop=(j == 5))
        tcp(gT_sb[:, 64 * dh: 64 * dh + 64], gp[:])
    s_ps = pa.tile([2 * P, D], f32, tag="pa")
    for kt in range(2):
        mm(s_ps[:], _r(gT_sb[:, 64 * kt: 64 * kt + 64]),
           _r(vw_sb[:, 256 * kt: 256 * kt + 256]),
           start=(kt == 0), stop=(kt == 1))
    tcp(s_sb[:], s_ps[:])

    # ---------------- retrieved^T = V^T s + S^T Q ----------------
    retr = []
    retr_cross = []
    for dh in range(2):
        rp = pb.tile([128, 512], f32, tag="pb")
        mm(rp[:, 0:256], _r(vodd_sb[:, 128 * dh: 128 * dh + 128]),
           _r(ss0_sb[:, 0:256]), start=True, stop=True)
        mm(rp[:, 256:512], _r(vodd_sb[:, 128 * dh: 128 * dh + 128]),
           _r(ss0_sb[:, 256:512]), start=True, stop=False)
        mm(rp[:, 256:512], _r(vodd_sb[:, 256 + 128 * dh: 256 + 128 * dh + 128]),
           _r(ss1_sb[:]), start=False, stop=True)
        cp = pa.tile([128, 512], f32, tag="pa")
        mm(cp[:], _r(s_sb[:, 128 * dh: 128 * dh + 128]), _r(qb_sb[:]),
           start=True, stop=True)
        nc.scalar.copy(cross_sb[:, 512 * dh: 512 * dh + 512], cp[:])
        retr.append(rp)
        retr_cross.append(cross_sb[:, 512 * dh: 512 * dh + 512])

    # ---------------- LN1 (folded) ----------------
    def ln_stats(src_ps, src_sb, sqt, stp, msqp, var_sb, m2v_sb,
                 mean_sb, badd, src_add=None):
        # copies src psum -> src_sb (bf16, + optional per-partition bias /
        # cross add), squares, stats; produces the mean row (bf16) and the
        # var row (+eps).
        for dh in range(2):
            sl = slice(512 * dh, 512 * dh + 512)
            if src_add is not None:
                tt_(src_sb[:, sl], src_ps[dh][:], src_add[dh], OP.add)
            elif badd is None:
                tcp(src_sb[:, sl], src_ps[dh][:])
            else:
                nc.vector.tensor_scalar(
                    out=src_sb[:, sl], in0=src_ps[dh][:],
                    scalar1=badd[:, dh: dh + 1], scalar2=None, op0=OP.add)
            act(sqt[:, sl], src_sb[:, sl], AF.Square, bias=zero128_sb[:])
        for kt in range(2):
            mm(stp[0:1, :], invdb_sb[:, 0:1],
               src_sb[:, 512 * kt: 512 * kt + 512],
               start=(kt == 0), stop=(kt == 1))
        for kt in range(2):
            mm(msqp[0:1, :], _r(invd_sb[:, 0:1]),
               _r(sqt[:, 512 * kt: 512 * kt + 512]),
               start=(kt == 0), stop=(kt == 1))
        tcp(mean_sb[:], stp[0:1, :])
        act(m2v_sb[:], stp[0:1, :], AF.Square, bias=zero1_sb[:])
        nc.vector.scalar_tensor_tensor(
            out=var_sb[:], in0=msqp[0:1, :], scalar=1e-5, in1=m2v_sb[:],
            op0=OP.add, op1=OP.subtract)

    def ln_rstd(var_sb, stdc, rstdc):
        # var row -> columns [128,4]; rstd = rsqrt(var) via magic-seed
        # Newton, all DVE, no ACT table loads.
        vc = pt.tile([128, 4], f32, tag="ptr")
        for j in range(4):
            nc.tensor.transpose(vc[:, j: j + 1],
                                var_sb[0:1, 128 * j: 128 * j + 128],
                                eyef_sb[0:1, 0:1])
        i32 = mybir.dt.int32
        nc.vector.tensor_copy(stdc[:], vc[:])
        nc.vector.tensor_scalar(out=rstdc[:].bitcast(i32),
                                in0=stdc[:].bitcast(i32), scalar1=1,
                                scalar2=None, op0=OP.logical_shift_right)
        nc.vector.tensor_scalar(out=rstdc[:].bitcast(i32),
                                in0=rstdc[:].bitcast(i32), scalar1=-1,
                                scalar2=0x5F3759DF, op0=OP.mult, op1=OP.add)
        nwt = pa.tile([128, 4], f32, tag="pa2", bufs=1)
        for _ in range(2):
            tt_(nwt[:], rstdc[:], rstdc[:], OP.mult)
            nc.vector.scalar_tensor_tensor(
                out=nwt[:], in0=nwt[:], scalar=-0.5, in1=stdc[:],
                op0=OP.mult, op1=OP.mult)
            nc.vector.tensor_scalar(out=nwt[:], in0=nwt[:], scalar1=1.5,
                                    scalar2=None, op0=OP.add)
            tt_(rstdc[:], rstdc[:], nwt[:], OP.mult)

    st1 = pa.tile([1, T], f32, tag="pa")
    ms1 = pa.tile([1, T], f32, tag="pa2", bufs=1)
    ln_stats(retr, r_sb, sq_sb, st1, ms1, var1_sb, m2_sb,
             mean1_sb, None, src_add=retr_cross)

    # W1 on raw r (rstd folded after the matmul):
    #   (W1g^T r - mean*c1) * rstd == W1g^T(LN1(r))
    # so the big matmuls run concurrently with the rstd row computation.
    def w1_block(m):
        hp = pb.tile([128, 512], f32, tag="pb")
        for kt in range(2):
            mm(hp[:], w1g_sb[:, 512 * kt + 128 * m: 512 * kt + 128 * m + 128],
               r_sb[:, 512 * kt: 512 * kt + 512],
               start=(kt == 0), stop=False)
        mm(hp[:], c1n_sb[0:1, 128 * m: 128 * m + 128], mean1_sb[:],
           start=False, stop=True)
        return hp

    hps = [w1_block(0)]
    ln_rstd(var1_sb, stdc_sb, rstdc_sb)
    hps.append(w1_block(1))
    # rstd1 broadcast row [128, T]
    rr = pa.tile([1, T], f32, tag="pa")
    for j in range(4):
        nc.tensor.transpose(rr[0:1, 128 * j: 128 * j + 128],
                            rstdc_sb[:, j: j + 1], eyef_sb[:])
    tcp(rstd1_sb[:], rr[:])
    rb1 = pb.tile([128, 512], f32, tag="pb")
    mm(rb1[:], _r(ones_sb[:]), _r(rstd1_sb[:]), start=True, stop=True)
    nc.scalar.copy(rb1s_sb[:], rb1[:])
    for m in range(4):
        if m >= len(hps):
            hps.append(w1_block(m))
        hp = hps[m]
        tt_(hi_sb[:, 512 * m: 512 * m + 512], hp[:], rb1s_sb[:], OP.mult)
        act(h_sb[:, 512 * m: 512 * m + 512],
            hi_sb[:, 512 * m: 512 * m + 512], AF.Gelu,
            bias=cb1_sb[:, m: m + 1])

    # ---------------- W2 -> refined ----------------
    ref = []
    for dh in range(2):
        fp = pb.tile([128, 512], f32, tag="pb")
        for kt in range(4):
            mm(fp[:], w2_sb[:, 256 * kt + 128 * dh: 256 * kt + 128 * dh + 128],
               h_sb[:, 512 * kt: 512 * kt + 512],
               start=(kt == 0), stop=(kt == 3))
        ref.append(fp)

    # ---------------- LN2 (folded) ----------------
    st2 = pa.tile([1, T], f32, tag="pa")
    ms2 = pa.tile([1, T], f32, tag="pa2", bufs=1)
    ln_stats(ref, f_sb, sq2_sb, st2, ms2, var2_sb, m2b_sb,
             mean2_sb, b2c_sb)

    # Wo (token-major) + rank-1 mean fix; rstd2 is per-partition in
    # token-major space so the LN2 apply fuses into the residual op.
    def wo_block(tm):
        op = pa.tile([128, D], f32, tag="pa")
        for dh in range(2):
            mm(op[:], f_sb[:, 512 * dh + 128 * tm: 512 * dh + 128 * tm + 128],
               wog_sb[:, 256 * dh: 256 * dh + 256],
               start=(dh == 0), stop=False)
        mm(op[:], mean2_sb[0:1, 128 * tm: 128 * tm + 128], c1on_sb[:],
           start=False, stop=True)
        return op

    ops = [wo_block(0)]
    ln_rstd(var2_sb, stdc2_sb, rstdc2_sb)
    for tm in range(4):
        if tm >= len(ops):
            ops.append(wo_block(tm))
        nc.vector.scalar_tensor_tensor(
            out=out_sb[:, 256 * tm: 256 * tm + 256], in0=ops[tm][:],
            scalar=rstdc2_sb[:, tm: tm + 1],
            in1=xc_sb[:, 256 * tm: 256 * tm + 256],
            op0=OP.mult, op1=OP.add)
        dma(outc[128 * tm: 128 * tm + 128, :],
            out_sb[:, 256 * tm: 256 * tm + 256])



    pb.release()
    pa.release()
    pt.release()
    sb.release()


_CACHE = {}


def _get_nc():
    if "nc" not in _CACHE:
        _CACHE["nc"] = _build()
    return _CACHE["nc"]


def _bf(a):
    return np.asarray(a, np.float32).astype(ml_dtypes.bfloat16)


def kernel(**inputs):
    x = np.asarray(inputs["x"], np.float32)
    key_W = np.asarray(inputs["key_W"], np.float32)
    key_b = np.asarray(inputs["key_b"], np.float32)
    val_W = np.asarray(inputs["val_W"], np.float32)
    val_b = np.asarray(inputs["val_b"], np.float32)
    ln1_g = np.asarray(inputs["ln1_g"], np.float32)
    ln1_b = np.asarray(inputs["ln1_b"], np.float32)
    W1 = np.asarray(inputs["W1"], np.float32)
    b1 = np.asarray(inputs["b1"], np.float32)
    W2 = np.asarray(inputs["W2"], np.float32)
    b2 = np.asarray(inputs["b2"], np.float32)
    ln2_g = np.asarray(inputs["ln2_g"], np.float32)
    ln2_b = np.asarray(inputs["ln2_b"], np.float32)
    Wo = np.asarray(inputs["Wo"], np.float32)
    bo = np.asarray(inputs["bo"], np.float32)

    # these are identically zero for this module; the kernel folds them out
    assert np.allclose(val_b, 0.0), "nonzero val_b unsupported"
    assert np.allclose(bo + ln2_b @ Wo, 0.0), "nonzero output bias unsupported"

    w1g = ln1_g[:, None] * W1
    wog = ln2_g[:, None] * Wo
    shared = {
        "kw": key_W, "kwb": _bf(key_W), "kbc": key_b.reshape(P, 1),
        "vw": val_W,
        "w1g": _bf(w1g), "c1n": _bf(-w1g.sum(0, keepdims=True)),
        "cb1": (b1 + ln1_b @ W1).reshape(H, 1),
        "w2": _bf(W2), "b2c": b2.reshape(D, 1),
        "wog": _bf(wog), "c1on": _bf(-wog.sum(0, keepdims=True)),
        "m0": (np.arange(1, 256, 2)[:, None] <=
               np.arange(256)[None, :]).astype(np.float32),
        "eyef": np.eye(128, dtype=np.float32),
        "onesr": np.ones((1, 128), np.float32),
        "invd": np.full((128, 1), 1.0 / D, np.float32),
        "invdb": _bf(np.full((128, 1), 1.0 / D, np.float32)),
        "eyeb": _bf(np.eye(128)),
    }
    in_maps = []
    for i in range(N_CORES):
        b, c = divmod(i, 4)
        l0 = c * T
        npairs = l0 // 2
        xpe = np.zeros((PRE, D), np.float32)
        xpo = np.zeros((PRE, D), np.float32)
        if npairs:
            xpe[:npairs] = x[b, 0:l0 - 1:2]
            xpo[:npairs] = x[b, 1:l0:2]
        in_maps.append({
            "xc": np.ascontiguousarray(x[b, l0:l0 + T]),
            "xpe": _bf(xpe), "xpo": _bf(xpo), **shared,
        })

    nc = _get_nc()
    res = run_bass_kernel_spmd(nc, in_maps, core_ids=list(range(N_CORES)),
                               **_CACHE.get("run_kwargs", {}))
    _CACHE["last_result"] = res
    out = np.empty((B, L, D), np.float32)
    for i in range(N_CORES):
        b, c = divmod(i, 4)
        out[b, c * T:(c + 1) * T] = res.results[i]["outc"]
    return out
